# revision 2
# baseline (speedup 1.0000x reference)
"""Trainium2 Bass kernel for DiversityInjection (MoE-style per-agent low-rank
perturbation + LayerNorm).

Strategy: expert-parallel over the 256 agents. The host routes tokens to the
core that owns their agent (MoE dispatch done host-side), packs them into
fixed-capacity per-agent slots, and each core runs dense per-slot matmuls:

    intT[r, t] = sum_h U[a][h, r] * h[t, h]      (8 chunk-matmuls, contract 1024)
    pert[t, j] = sum_r intT[r, t] * (alpha*V[a])[r, j]
    out = LayerNorm(h + pert)  (per-token, fused bn_stats + activation)

Slots are processed in pairs so the LayerNorm stage operates on full
[128, 1024] tiles. The padded output is scattered back to the original token
order on the host.
"""

import os
import sys

for _p in ("/opt/trn_rl_repo", "/root/.axon_site/_ro/trn_rl_repo"):
    if os.path.isdir(_p) and _p not in sys.path:
        sys.path.insert(0, _p)

import numpy as np

N_CORES = 8
CAP = 64           # tokens per slot (per-agent capacity)
ALPHA_MAX = 5.0
LN_EPS = 1e-5

_PROGRAM_CACHE = {}


def _reference_numpy(h, log_alpha, ln_gamma, ln_beta, projection_u, projection_v,
                     agent_ids):
    """Fallback pure-numpy implementation (used only if packing does not fit)."""
    num_agents = projection_u.shape[0]
    ids = agent_ids % num_agents
    alpha = min(np.exp(np.float32(log_alpha)), np.float32(ALPHA_MAX))
    out = np.empty_like(h)
    for a in range(num_agents):
        m = ids == a
        if not m.any():
            continue
        hb = h[m]
        pert = (hb @ projection_u[a]) @ projection_v[a]
        out[m] = hb + alpha * pert
    mean = out.mean(-1, keepdims=True, dtype=np.float64)
    var = out.var(-1, keepdims=True, dtype=np.float64)
    out = (out - mean) / np.sqrt(var + LN_EPS)
    return (out * ln_gamma + ln_beta).astype(h.dtype)


def _build_program(nslot, hidden, rank, use_gamma, use_beta):
    """Build the per-core Bass program. Same program runs SPMD on all 8 cores."""
    from contextlib import ExitStack

    import concourse.bacc as bacc
    import concourse.mybir as mybir
    import concourse.tile as tile

    assert hidden % 1024 == 0 and hidden == 1024
    assert nslot % 2 == 0
    npair = nslot // 2
    nchunk = hidden // 128  # contract chunks for the first matmul

    nc = bacc.Bacc("TRN2", target_bir_lowering=False, debug=False)

    u_d = nc.dram_tensor("u_sw", (nslot, 128, nchunk * rank), mybir.dt.float32,
                         kind="ExternalInput")
    v_d = nc.dram_tensor("v_sw", (nslot, rank, hidden), mybir.dt.float32,
                         kind="ExternalInput")
    hT_d = nc.dram_tensor("hT_sw", (npair, nchunk, 128, 2 * CAP), mybir.dt.float32,
                          kind="ExternalInput")
    h_d = nc.dram_tensor("h_pk", (npair, 2 * CAP, hidden), mybir.dt.float32,
                         kind="ExternalInput")
    gb_d = None
    if use_gamma or use_beta:
        gb_d = nc.dram_tensor("gb_rep", (2, 128, hidden), mybir.dt.float32,
                              kind="ExternalInput")
    out_d = nc.dram_tensor("out_pk", (npair, 2 * CAP, hidden), mybir.dt.float32,
                           kind="ExternalOutput")

    with tile.TileContext(nc) as tc, ExitStack() as ctx:
        upool = ctx.enter_context(tc.tile_pool(name="u", bufs=4))
        vpool = ctx.enter_context(tc.tile_pool(name="v", bufs=4))
        htpool = ctx.enter_context(tc.tile_pool(name="hT", bufs=3))
        hpool = ctx.enter_context(tc.tile_pool(name="h", bufs=3))
        ipool = ctx.enter_context(tc.tile_pool(name="intT", bufs=4))
        spool = ctx.enter_context(tc.tile_pool(name="stats", bufs=6))
        xpool = ctx.enter_context(tc.tile_pool(name="x", bufs=3))
        opool = ctx.enter_context(tc.tile_pool(name="o", bufs=3))
        cpool = ctx.enter_context(tc.tile_pool(name="const", bufs=1))
        p1pool = ctx.enter_context(tc.tile_pool(name="psum1", bufs=4, space="PSUM"))
        p2pool = ctx.enter_context(tc.tile_pool(name="psum2", bufs=2, space="PSUM"))

        eps_t = cpool.tile([128, 1], mybir.dt.float32)
        nc.vector.memset(eps_t[:], LN_EPS)
        gb_t = None
        if gb_d is not None:
            gb_t = cpool.tile([128, 2 * hidden], mybir.dt.float32)
            nc.sync.dma_start(
                gb_t[:].rearrange("p (g f) -> g p f", g=2), gb_d.ap())

        for p in range(npair):
            hT_t = htpool.tile([128, nchunk * 2 * CAP], mybir.dt.float32)
            nc.sync.dma_start(
                hT_t[:].rearrange("p (c t) -> p c t", c=nchunk),
                hT_d[p].rearrange("c p t -> p c t"))
            h_t = hpool.tile([128, hidden], mybir.dt.float32)
            nc.sync.dma_start(h_t[:], h_d[p])

            psum2 = p2pool.tile([128, hidden], mybir.dt.float32)
            for s in range(2):
                slot = 2 * p + s
                u_t = upool.tile([128, nchunk * rank], mybir.dt.float32)
                nc.sync.dma_start(u_t[:], u_d[slot])
                v_t = vpool.tile([rank, hidden], mybir.dt.float32)
                nc.sync.dma_start(v_t[:], v_d[slot])

                psum1 = p1pool.tile([rank, CAP], mybir.dt.float32)
                for c in range(nchunk):
                    nc.tensor.matmul(
                        psum1[:],
                        u_t[:, c * rank:(c + 1) * rank],
                        hT_t[:, c * 2 * CAP + s * CAP: c * 2 * CAP + (s + 1) * CAP],
                        start=(c == 0), stop=(c == nchunk - 1),
                    )
                intT = ipool.tile([rank, CAP], mybir.dt.float32)
                nc.scalar.copy(intT[:], psum1[:])
                for q in range(hidden // 512):
                    nc.tensor.matmul(
                        psum2[s * CAP:(s + 1) * CAP, q * 512:(q + 1) * 512],
                        intT[:],
                        v_t[:, q * 512:(q + 1) * 512],
                        start=True, stop=True,
                    )

            # x = h + pert  (DVE reads PSUM + SBUF)
            x_t = xpool.tile([128, hidden], mybir.dt.float32)
            nc.vector.tensor_add(x_t[:], psum2[:], h_t[:])
            # LayerNorm stats
            stats = spool.tile([128, 6 * (hidden // 512)], mybir.dt.float32)
            for q in range(hidden // 512):
                nc.vector.bn_stats(stats[:, q * 6:(q + 1) * 6],
                                   x_t[:, q * 512:(q + 1) * 512])
            aggr = spool.tile([128, 2], mybir.dt.float32)
            nc.vector.bn_aggr(aggr[:], stats[:].rearrange("p (c s) -> p c s", s=3))
            std = spool.tile([128, 1], mybir.dt.float32)
            nc.scalar.activation(std[:], aggr[:, 1:2],
                                 mybir.ActivationFunctionType.Sqrt,
                                 bias=eps_t[:, 0:1])
            rstd = spool.tile([128, 1], mybir.dt.float32)
            nc.vector.reciprocal(rstd[:], std[:])
            nmr = spool.tile([128, 1], mybir.dt.float32)
            nc.vector.scalar_tensor_tensor(nmr[:], aggr[:, 0:1], -1.0, rstd[:],
                                           mybir.AluOpType.mult,
                                           mybir.AluOpType.mult)
            o_t = opool.tile([128, hidden], mybir.dt.float32)
            nc.scalar.activation(o_t[:], x_t[:],
                                 mybir.ActivationFunctionType.Identity,
                                 bias=nmr[:, 0:1], scale=rstd[:, 0:1])
            if use_gamma:
                nc.vector.tensor_mul(o_t[:], o_t[:], gb_t[:, 0:hidden])
            if use_beta:
                nc.vector.tensor_add(o_t[:], o_t[:], gb_t[:, hidden:2 * hidden])
            nc.sync.dma_start(out_d[p], o_t[:])

    nc.finalize()
    return nc


def _get_program(nslot, hidden, rank, use_gamma, use_beta):
    key = (nslot, hidden, rank, use_gamma, use_beta)
    if key not in _PROGRAM_CACHE:
        _PROGRAM_CACHE[key] = _build_program(nslot, hidden, rank, use_gamma,
                                             use_beta)
    return _PROGRAM_CACHE[key]


def kernel(h, log_alpha, ln_gamma, ln_beta, projection_u, projection_v,
           agent_ids):
    h = np.asarray(h, dtype=np.float32)
    projection_u = np.asarray(projection_u, dtype=np.float32)
    projection_v = np.asarray(projection_v, dtype=np.float32)
    ln_gamma = np.asarray(ln_gamma, dtype=np.float32)
    ln_beta = np.asarray(ln_beta, dtype=np.float32)
    ids_raw = np.asarray(agent_ids)
    log_alpha = np.float32(np.asarray(log_alpha))

    B, H = h.shape
    A, H2, R = projection_u.shape
    ids = (ids_raw.astype(np.int64) % A).astype(np.int32)

    if H != 1024 or H2 != H or R > 128 or projection_v.shape != (A, R, H):
        return _reference_numpy(h, log_alpha, ln_gamma, ln_beta, projection_u,
                                projection_v, agent_ids)

    alpha = np.float32(min(np.exp(log_alpha), np.float32(ALPHA_MAX)))
    use_gamma = not np.all(ln_gamma == 1.0)
    use_beta = not np.all(ln_beta == 0.0)

    # ---- host-side MoE dispatch: sort tokens by agent, build capacity slots
    order = np.argsort(ids, kind="stable").astype(np.int64)
    counts = np.bincount(ids, minlength=A)
    starts = np.zeros(A + 1, np.int64)
    np.cumsum(counts, out=starts[1:])

    slot_agent = []   # agent id per slot
    slot_rows = []    # (start, n) into `order` per slot
    for a in range(A):
        n = int(counts[a])
        s = int(starts[a])
        while n > 0:
            take = min(n, CAP)
            slot_agent.append(a)
            slot_rows.append((s, take))
            s += take
            n -= take
        if counts[a] == 0:
            pass
    total_slots = len(slot_agent)
    nslot = -(-total_slots // N_CORES)
    nslot = max(nslot, 2)
    if nslot % 2:
        nslot += 1
    if nslot > 64:  # way off the expected distribution; play it safe
        return _reference_numpy(h, log_alpha, ln_gamma, ln_beta, projection_u,
                                projection_v, agent_ids)
    # pad with dummy slots (agent 0, zero tokens)
    while len(slot_agent) < nslot * N_CORES:
        slot_agent.append(0)
        slot_rows.append((0, 0))
    slot_agent = np.asarray(slot_agent, np.int64)

    npair = nslot // 2
    nchunk = H // 128

    # row_idx: global token index feeding each padded row (clamped for pads)
    nrows = nslot * CAP
    row_idx = np.zeros((N_CORES, nrows), np.int64)
    row_valid = np.zeros((N_CORES, nrows), bool)
    for j, (s, n) in enumerate(slot_rows):
        core, sl = divmod(j, nslot)
        r0 = sl * CAP
        if n:
            row_idx[core, r0:r0 + n] = order[s:s + n]
            row_valid[core, r0:r0 + n] = True

    h_pk = h[row_idx]                                   # [8, nrows, H]
    h_pk = np.ascontiguousarray(h_pk).reshape(N_CORES, npair, 2 * CAP, H)
    hT_sw = np.ascontiguousarray(
        h_pk.reshape(N_CORES, npair, 2 * CAP, nchunk, 128)
        .transpose(0, 1, 3, 4, 2))                      # [8, npair, c, 128, 2C]

    sa = slot_agent.reshape(N_CORES, nslot)
    u_sw = np.ascontiguousarray(
        projection_u[sa]                                # [8, ns, H, R]
        .reshape(N_CORES, nslot, nchunk, 128, R)
        .transpose(0, 1, 3, 2, 4)                       # [8, ns, 128, c, R]
    ).reshape(N_CORES, nslot, 128, nchunk * R)
    v_sw = np.ascontiguousarray(alpha * projection_v[sa])  # [8, ns, R, H]

    in_maps = []
    for core in range(N_CORES):
        m = {
            "u_sw": u_sw[core],
            "v_sw": v_sw[core],
            "hT_sw": hT_sw[core],
            "h_pk": h_pk[core],
        }
        if use_gamma or use_beta:
            m["gb_rep"] = np.ascontiguousarray(
                np.stack([np.broadcast_to(ln_gamma, (128, H)),
                          np.broadcast_to(ln_beta, (128, H))]))
        in_maps.append(m)

    nc = _get_program(nslot, H, R, use_gamma, use_beta)

    from concourse.bass_utils import run_bass_kernel_spmd
    res = run_bass_kernel_spmd(nc, in_maps, list(range(N_CORES)))

    out = np.empty_like(h)
    for core in range(N_CORES):
        o = np.asarray(res.results[core]["out_pk"]).reshape(nrows, H)
        out[row_idx[core][row_valid[core]]] = o[row_valid[core]]
    return out


# revision 3
# speedup vs baseline: 1.6879x; 1.6879x over previous
"""Trainium2 Bass kernel for DiversityInjection (MoE-style per-agent low-rank
perturbation + LayerNorm).

Strategy: expert-parallel over the 256 agents. The host routes tokens to the
core that owns their agent (MoE dispatch done host-side), packs them into
fixed-capacity per-agent slots (CAP=64), and each core runs dense per-slot
matmuls. To keep the tensor engine efficient the matmuls are batched:

  mm1 (4 slots at once): psum1[128, 256] = [U_a|U_b|U_c|U_d]^T @ hT4
        8 contract chunks of 128; useful output = 4 diagonal [32, 64] blocks
  mm2 (2 slots at once, block-diag): psum2[128, 512] =
        [[intT_A, 0], [0, intT_B]]^T(64x128) @ [V_A; V_B](64x512)
  out = LayerNorm(h + pert) fused via bn_stats + scalar activation

The padded output is scattered back to original token order on the host.
"""

import os
import sys

for _p in ("/opt/trn_rl_repo", "/root/.axon_site/_ro/trn_rl_repo"):
    if os.path.isdir(_p) and _p not in sys.path:
        sys.path.insert(0, _p)

import numpy as np

N_CORES = 8
CAP = 64           # tokens per slot (per-agent capacity)
ALPHA_MAX = 5.0
LN_EPS = 1e-5
VARIANT = os.environ.get("BASS_KERNEL_VARIANT", "b32")

_PROGRAM_CACHE = {}


def _reference_numpy(h, log_alpha, ln_gamma, ln_beta, projection_u, projection_v,
                     agent_ids):
    """Fallback pure-numpy implementation (used only if packing does not fit)."""
    num_agents = projection_u.shape[0]
    ids = agent_ids % num_agents
    alpha = min(np.exp(np.float32(log_alpha)), np.float32(ALPHA_MAX))
    out = np.empty_like(h)
    for a in range(num_agents):
        m = ids == a
        if not m.any():
            continue
        hb = h[m]
        pert = (hb @ projection_u[a]) @ projection_v[a]
        out[m] = hb + alpha * pert
    mean = out.mean(-1, keepdims=True, dtype=np.float64)
    var = out.var(-1, keepdims=True, dtype=np.float64)
    out = (out - mean) / np.sqrt(var + LN_EPS)
    return (out * ln_gamma + ln_beta).astype(h.dtype)


def _build_program(nslot, hidden, rank, use_gamma, use_beta, variant):
    """Build the per-core Bass program. Same program runs SPMD on all 8 cores."""
    from contextlib import ExitStack

    import concourse.bacc as bacc
    import concourse.mybir as mybir
    import concourse.tile as tile

    assert hidden == 1024 and rank == 32
    assert nslot % 4 == 0
    npair = nslot // 2
    ngroup = nslot // 4
    nchunk = hidden // 128

    mmdt = mybir.dt.float32r if variant == "b32r" else mybir.dt.float32

    nc = bacc.Bacc("TRN2", target_bir_lowering=False, debug=False)

    u4_d = nc.dram_tensor("u4_sw", (ngroup, 128, nchunk * 4 * rank), mmdt,
                          kind="ExternalInput")
    v_d = nc.dram_tensor("v_sw", (npair, 2 * rank, hidden), mmdt,
                         kind="ExternalInput")
    hT_d = nc.dram_tensor("hT_sw", (ngroup, 128, nchunk * 4 * CAP), mmdt,
                          kind="ExternalInput")
    h_d = nc.dram_tensor("h_pk", (npair, 2 * CAP, hidden), mybir.dt.float32,
                         kind="ExternalInput")
    gb_d = None
    if use_gamma or use_beta:
        gb_d = nc.dram_tensor("gb_rep", (2, 128, hidden), mybir.dt.float32,
                              kind="ExternalInput")
    out_d = nc.dram_tensor("out_pk", (npair, 2 * CAP, hidden), mybir.dt.float32,
                           kind="ExternalOutput")

    with tile.TileContext(nc) as tc, ExitStack() as ctx:
        upool = ctx.enter_context(tc.tile_pool(name="u", bufs=3))
        vpool = ctx.enter_context(tc.tile_pool(name="v", bufs=4))
        htpool = ctx.enter_context(tc.tile_pool(name="hT", bufs=3))
        hpool = ctx.enter_context(tc.tile_pool(name="h", bufs=4))
        bpool = ctx.enter_context(tc.tile_pool(name="blk", bufs=4))
        spool = ctx.enter_context(tc.tile_pool(name="stats", bufs=8))
        xpool = ctx.enter_context(tc.tile_pool(name="x", bufs=4))
        opool = ctx.enter_context(tc.tile_pool(name="o", bufs=4))
        cpool = ctx.enter_context(tc.tile_pool(name="const", bufs=1))
        p1pool = ctx.enter_context(tc.tile_pool(name="psum1", bufs=2, space="PSUM"))
        p2pool = ctx.enter_context(tc.tile_pool(name="psum2", bufs=3, space="PSUM"))

        eps_t = cpool.tile([128, 1], mybir.dt.float32)
        nc.vector.memset(eps_t[:], LN_EPS)
        gb_t = None
        if gb_d is not None:
            gb_t = cpool.tile([128, 2 * hidden], mybir.dt.float32)
            nc.sync.dma_start(
                gb_t[:].rearrange("p (g f) -> g p f", g=2), gb_d.ap())

        for g in range(ngroup):
            u4_t = upool.tile([128, nchunk * 4 * rank], mmdt)
            nc.sync.dma_start(u4_t[:], u4_d[g])
            hT_t = htpool.tile([128, nchunk * 4 * CAP], mmdt)
            nc.sync.dma_start(hT_t[:], hT_d[g])

            psum1 = p1pool.tile([128, 4 * CAP], mybir.dt.float32)
            for c in range(nchunk):
                nc.tensor.matmul(
                    psum1[:],
                    u4_t[:, c * 4 * rank:(c + 1) * 4 * rank],
                    hT_t[:, c * 4 * CAP:(c + 1) * 4 * CAP],
                    start=(c == 0), stop=(c == nchunk - 1),
                )

            for pp in range(2):          # pairs within the group
                p = 2 * g + pp
                # block-diag [2*rank, 128] lhsT for the pair's two slots
                blk = bpool.tile([2 * rank, 2 * CAP], mmdt)
                nc.gpsimd.memset(blk[0:rank, CAP:2 * CAP], 0.0)
                nc.gpsimd.memset(blk[rank:2 * rank, 0:CAP], 0.0)
                for s2 in range(2):      # slot within pair
                    s = 2 * pp + s2      # slot within group
                    nc.scalar.copy(
                        blk[s2 * rank:(s2 + 1) * rank, s2 * CAP:(s2 + 1) * CAP],
                        psum1[s * rank:(s + 1) * rank, s * CAP:(s + 1) * CAP])

                v2_t = vpool.tile([2 * rank, hidden], mmdt)
                nc.sync.dma_start(v2_t[:], v_d[p])
                h_t = hpool.tile([128, hidden], mybir.dt.float32)
                nc.sync.dma_start(h_t[:], h_d[p])

                psum2 = p2pool.tile([128, hidden], mybir.dt.float32)
                for q in range(hidden // 512):
                    nc.tensor.matmul(
                        psum2[:, q * 512:(q + 1) * 512],
                        blk[:],
                        v2_t[:, q * 512:(q + 1) * 512],
                        start=True, stop=True,
                    )

                # x = h + pert  (DVE reads PSUM + SBUF)
                x_t = xpool.tile([128, hidden], mybir.dt.float32)
                nc.vector.tensor_add(x_t[:], psum2[:], h_t[:])
                stats = spool.tile([128, 6 * (hidden // 512)], mybir.dt.float32)
                for q in range(hidden // 512):
                    nc.vector.bn_stats(stats[:, q * 6:(q + 1) * 6],
                                       x_t[:, q * 512:(q + 1) * 512])
                aggr = spool.tile([128, 2], mybir.dt.float32)
                nc.vector.bn_aggr(aggr[:],
                                  stats[:].rearrange("p (c s) -> p c s", s=3))
                std = spool.tile([128, 1], mybir.dt.float32)
                nc.scalar.activation(std[:], aggr[:, 1:2],
                                     mybir.ActivationFunctionType.Sqrt,
                                     bias=eps_t[:, 0:1])
                rstd = spool.tile([128, 1], mybir.dt.float32)
                nc.vector.reciprocal(rstd[:], std[:])
                nmr = spool.tile([128, 1], mybir.dt.float32)
                nc.vector.scalar_tensor_tensor(nmr[:], aggr[:, 0:1], -1.0,
                                               rstd[:],
                                               mybir.AluOpType.mult,
                                               mybir.AluOpType.mult)
                o_t = opool.tile([128, hidden], mybir.dt.float32)
                nc.scalar.activation(o_t[:], x_t[:],
                                     mybir.ActivationFunctionType.Identity,
                                     bias=nmr[:, 0:1], scale=rstd[:, 0:1])
                if use_gamma:
                    nc.vector.tensor_mul(o_t[:], o_t[:], gb_t[:, 0:hidden])
                if use_beta:
                    nc.vector.tensor_add(o_t[:], o_t[:],
                                         gb_t[:, hidden:2 * hidden])
                nc.sync.dma_start(out_d[p], o_t[:])

    nc.finalize()
    return nc


def _get_program(nslot, hidden, rank, use_gamma, use_beta, variant):
    key = (nslot, hidden, rank, use_gamma, use_beta, variant)
    if key not in _PROGRAM_CACHE:
        _PROGRAM_CACHE[key] = _build_program(nslot, hidden, rank, use_gamma,
                                             use_beta, variant)
    return _PROGRAM_CACHE[key]


def kernel(h, log_alpha, ln_gamma, ln_beta, projection_u, projection_v,
           agent_ids):
    h = np.asarray(h, dtype=np.float32)
    projection_u = np.asarray(projection_u, dtype=np.float32)
    projection_v = np.asarray(projection_v, dtype=np.float32)
    ln_gamma = np.asarray(ln_gamma, dtype=np.float32)
    ln_beta = np.asarray(ln_beta, dtype=np.float32)
    ids_raw = np.asarray(agent_ids)
    log_alpha = np.float32(np.asarray(log_alpha))

    B, H = h.shape
    A, H2, R = projection_u.shape
    ids = (ids_raw.astype(np.int64) % A).astype(np.int32)

    if H != 1024 or H2 != H or R != 32 or projection_v.shape != (A, R, H):
        return _reference_numpy(h, log_alpha, ln_gamma, ln_beta, projection_u,
                                projection_v, agent_ids)

    alpha = np.float32(min(np.exp(log_alpha), np.float32(ALPHA_MAX)))
    use_gamma = not np.all(ln_gamma == 1.0)
    use_beta = not np.all(ln_beta == 0.0)

    # ---- host-side MoE dispatch: sort tokens by agent, build capacity slots
    order = np.argsort(ids, kind="stable").astype(np.int64)
    counts = np.bincount(ids, minlength=A)
    starts = np.zeros(A + 1, np.int64)
    np.cumsum(counts, out=starts[1:])

    slot_agent = []   # agent id per slot
    slot_rows = []    # (start, n) into `order` per slot
    for a in range(A):
        n = int(counts[a])
        s = int(starts[a])
        while n > 0:
            take = min(n, CAP)
            slot_agent.append(a)
            slot_rows.append((s, take))
            s += take
            n -= take
    total_slots = len(slot_agent)
    nslot = -(-total_slots // N_CORES)
    nslot = max(nslot, 4)
    if nslot % 4:
        nslot += 4 - nslot % 4
    if nslot > 64:  # way off the expected distribution; play it safe
        return _reference_numpy(h, log_alpha, ln_gamma, ln_beta, projection_u,
                                projection_v, agent_ids)
    while len(slot_agent) < nslot * N_CORES:
        slot_agent.append(0)
        slot_rows.append((0, 0))
    slot_agent = np.asarray(slot_agent, np.int64)

    npair = nslot // 2
    ngroup = nslot // 4
    nchunk = H // 128

    # row_idx: global token index feeding each padded row (clamped for pads)
    nrows = nslot * CAP
    row_idx = np.zeros((N_CORES, nrows), np.int64)
    row_valid = np.zeros((N_CORES, nrows), bool)
    for j, (s, n) in enumerate(slot_rows):
        core, sl = divmod(j, nslot)
        r0 = sl * CAP
        if n:
            row_idx[core, r0:r0 + n] = order[s:s + n]
            row_valid[core, r0:r0 + n] = True

    h_pk = np.ascontiguousarray(h[row_idx]).reshape(N_CORES, npair, 2 * CAP, H)
    # hT per group: [p(128), c(8), t4(256)]
    hT_sw = np.ascontiguousarray(
        h_pk.reshape(N_CORES, ngroup, 4 * CAP, nchunk, 128)
        .transpose(0, 1, 4, 3, 2)).reshape(N_CORES, ngroup, 128,
                                           nchunk * 4 * CAP)

    sa = slot_agent.reshape(N_CORES, nslot)
    # u4: [g, p(128), c(8), s(4), r(32)]
    u4_sw = np.ascontiguousarray(
        projection_u[sa]                                  # [8, ns, H, R]
        .reshape(N_CORES, ngroup, 4, nchunk, 128, R)
        .transpose(0, 1, 4, 3, 2, 5)                      # [8, g, 128, c, 4, R]
    ).reshape(N_CORES, ngroup, 128, nchunk * 4 * R)
    v_sw = np.ascontiguousarray(alpha * projection_v[sa]).reshape(
        N_CORES, npair, 2 * R, H)

    in_maps = []
    for core in range(N_CORES):
        m = {
            "u4_sw": u4_sw[core],
            "v_sw": v_sw[core],
            "hT_sw": hT_sw[core],
            "h_pk": h_pk[core],
        }
        if use_gamma or use_beta:
            m["gb_rep"] = np.ascontiguousarray(
                np.stack([np.broadcast_to(ln_gamma, (128, H)),
                          np.broadcast_to(ln_beta, (128, H))]))
        in_maps.append(m)

    nc = _get_program(nslot, H, R, use_gamma, use_beta, VARIANT)

    from concourse.bass_utils import run_bass_kernel_spmd
    res = run_bass_kernel_spmd(nc, in_maps, list(range(N_CORES)))

    out = np.empty_like(h)
    for core in range(N_CORES):
        o = np.asarray(res.results[core]["out_pk"]).reshape(nrows, H)
        out[row_idx[core][row_valid[core]]] = o[row_valid[core]]
    return out


# revision 4
# speedup vs baseline: 1.8699x; 1.1078x over previous
"""Trainium2 Bass kernel for DiversityInjection (MoE-style per-agent low-rank
perturbation + LayerNorm).

Strategy: expert-parallel over the 256 agents. The host routes tokens to the
core that owns their agent (MoE dispatch done host-side), packs them into
fixed-capacity per-agent slots (CAP tokens), and each core runs dense batched
matmuls over groups of G=3 slots (126 tokens per group tile):

  mm1 (3 slots at once): psum1[96, 126] = [U_a|U_b|U_c]^T @ hT3
        8 contract chunks of 128; useful output = 3 diagonal [32, 42] blocks
  mm2 (3 slots at once, block-diag): psum2[126, 512] =
        blockdiag(intT_a, intT_b, intT_c)^T(96x126) @ [V_a; V_b; V_c](96x512)
  out = LayerNorm(h + pert) fused via bn_stats + scalar activation

The padded output is scattered back to original token order on the host.
"""

import os
import sys

for _p in ("/opt/trn_rl_repo", "/root/.axon_site/_ro/trn_rl_repo"):
    if os.path.isdir(_p) and _p not in sys.path:
        sys.path.insert(0, _p)

import numpy as np

N_CORES = 8
CAP = 42           # tokens per slot (per-agent capacity)
G = 3              # slots per group tile (G*CAP <= 128, G*rank <= 128)
ALPHA_MAX = 5.0
LN_EPS = 1e-5
VARIANT = os.environ.get("BASS_KERNEL_VARIANT", "t32")

_PROGRAM_CACHE = {}


def _reference_numpy(h, log_alpha, ln_gamma, ln_beta, projection_u, projection_v,
                     agent_ids):
    """Fallback pure-numpy implementation (used only if packing does not fit)."""
    num_agents = projection_u.shape[0]
    ids = agent_ids % num_agents
    alpha = min(np.exp(np.float32(log_alpha)), np.float32(ALPHA_MAX))
    out = np.empty_like(h)
    for a in range(num_agents):
        m = ids == a
        if not m.any():
            continue
        hb = h[m]
        pert = (hb @ projection_u[a]) @ projection_v[a]
        out[m] = hb + alpha * pert
    mean = out.mean(-1, keepdims=True, dtype=np.float64)
    var = out.var(-1, keepdims=True, dtype=np.float64)
    out = (out - mean) / np.sqrt(var + LN_EPS)
    return (out * ln_gamma + ln_beta).astype(h.dtype)


def _build_program(nslot, hidden, rank, use_gamma, use_beta, variant):
    """Build the per-core Bass program. Same program runs SPMD on all 8 cores."""
    from contextlib import ExitStack

    import concourse.bacc as bacc
    import concourse.mybir as mybir
    import concourse.tile as tile

    assert hidden == 1024 and rank == 32
    assert nslot % G == 0
    ngroup = nslot // G
    nchunk = hidden // 128
    T = G * CAP          # tokens per group tile (126)
    KR = G * rank        # stacked rank (96)

    mmdt = mybir.dt.float32r if variant.endswith("r") else mybir.dt.float32

    nc = bacc.Bacc("TRN2", target_bir_lowering=False, debug=False)

    u_d = nc.dram_tensor("u_sw", (ngroup, 128, nchunk * KR), mmdt,
                         kind="ExternalInput")
    v_d = nc.dram_tensor("v_sw", (ngroup, KR, hidden), mmdt,
                         kind="ExternalInput")
    hT_d = nc.dram_tensor("hT_sw", (ngroup, 128, nchunk * T), mmdt,
                          kind="ExternalInput")
    h_d = nc.dram_tensor("h_pk", (ngroup, T, hidden), mybir.dt.float32,
                         kind="ExternalInput")
    gb_d = None
    if use_gamma or use_beta:
        gb_d = nc.dram_tensor("gb_rep", (2, 128, hidden), mybir.dt.float32,
                              kind="ExternalInput")
    out_d = nc.dram_tensor("out_pk", (ngroup, T, hidden), mybir.dt.float32,
                           kind="ExternalOutput")

    with tile.TileContext(nc) as tc, ExitStack() as ctx:
        upool = ctx.enter_context(tc.tile_pool(name="u", bufs=3))
        vpool = ctx.enter_context(tc.tile_pool(name="v", bufs=3))
        htpool = ctx.enter_context(tc.tile_pool(name="hT", bufs=3))
        hpool = ctx.enter_context(tc.tile_pool(name="h", bufs=4))
        bpool = ctx.enter_context(tc.tile_pool(name="blk", bufs=3))
        spool = ctx.enter_context(tc.tile_pool(name="stats", bufs=8))
        xpool = ctx.enter_context(tc.tile_pool(name="x", bufs=4))
        opool = ctx.enter_context(tc.tile_pool(name="o", bufs=4))
        cpool = ctx.enter_context(tc.tile_pool(name="const", bufs=1))
        p1pool = ctx.enter_context(tc.tile_pool(name="psum1", bufs=2, space="PSUM"))
        p2pool = ctx.enter_context(tc.tile_pool(name="psum2", bufs=3, space="PSUM"))

        eps_t = cpool.tile([128, 1], mybir.dt.float32)
        nc.vector.memset(eps_t[:], LN_EPS)
        gb_t = None
        if gb_d is not None:
            gb_t = cpool.tile([128, 2 * hidden], mybir.dt.float32)
            nc.sync.dma_start(
                gb_t[:].rearrange("p (g f) -> g p f", g=2), gb_d.ap())

        for g in range(ngroup):
            u_t = upool.tile([128, nchunk * KR], mmdt)
            nc.sync.dma_start(u_t[:], u_d[g])
            hT_t = htpool.tile([128, nchunk * T], mmdt)
            nc.sync.dma_start(hT_t[:], hT_d[g])

            psum1 = p1pool.tile([KR, T], mybir.dt.float32)
            for c in range(nchunk):
                nc.tensor.matmul(
                    psum1[:],
                    u_t[:, c * KR:(c + 1) * KR],
                    hT_t[:, c * T:(c + 1) * T],
                    start=(c == 0), stop=(c == nchunk - 1),
                )

            # block-diag [KR, T] lhsT: diagonal [rank, CAP] blocks from psum1
            blk = bpool.tile([KR, T], mmdt)
            for s in range(G):
                for s2 in range(G):
                    if s == s2:
                        nc.scalar.copy(
                            blk[s * rank:(s + 1) * rank,
                                s2 * CAP:(s2 + 1) * CAP],
                            psum1[s * rank:(s + 1) * rank,
                                  s2 * CAP:(s2 + 1) * CAP])
                    else:
                        nc.gpsimd.memset(
                            blk[s * rank:(s + 1) * rank,
                                s2 * CAP:(s2 + 1) * CAP], 0.0)

            v_t = vpool.tile([KR, hidden], mmdt)
            nc.sync.dma_start(v_t[:], v_d[g])
            h_t = hpool.tile([128, hidden], mybir.dt.float32)
            nc.sync.dma_start(h_t[0:T, :], h_d[g])

            psum2 = p2pool.tile([128, hidden], mybir.dt.float32)
            for q in range(hidden // 512):
                nc.tensor.matmul(
                    psum2[0:T, q * 512:(q + 1) * 512],
                    blk[:],
                    v_t[:, q * 512:(q + 1) * 512],
                    start=True, stop=True,
                )

            # x = h + pert  (DVE reads PSUM + SBUF)
            x_t = xpool.tile([128, hidden], mybir.dt.float32)
            nc.vector.tensor_add(x_t[0:T, :], psum2[0:T, :], h_t[0:T, :])
            stats = spool.tile([128, 6 * (hidden // 512)], mybir.dt.float32)
            for q in range(hidden // 512):
                nc.vector.bn_stats(stats[0:T, q * 6:(q + 1) * 6],
                                   x_t[0:T, q * 512:(q + 1) * 512])
            aggr = spool.tile([128, 2], mybir.dt.float32)
            nc.vector.bn_aggr(aggr[0:T, :],
                              stats[0:T, :].rearrange("p (c s) -> p c s", s=3))
            std = spool.tile([128, 1], mybir.dt.float32)
            nc.scalar.activation(std[0:T, :], aggr[0:T, 1:2],
                                 mybir.ActivationFunctionType.Sqrt,
                                 bias=eps_t[0:T, 0:1])
            rstd = spool.tile([128, 1], mybir.dt.float32)
            nc.vector.reciprocal(rstd[0:T, :], std[0:T, :])
            nmr = spool.tile([128, 1], mybir.dt.float32)
            nc.vector.scalar_tensor_tensor(nmr[0:T, :], aggr[0:T, 0:1], -1.0,
                                           rstd[0:T, :],
                                           mybir.AluOpType.mult,
                                           mybir.AluOpType.mult)
            o_t = opool.tile([128, hidden], mybir.dt.float32)
            nc.scalar.activation(o_t[0:T, :], x_t[0:T, :],
                                 mybir.ActivationFunctionType.Identity,
                                 bias=nmr[0:T, 0:1], scale=rstd[0:T, 0:1])
            if use_gamma:
                nc.vector.tensor_mul(o_t[0:T, :], o_t[0:T, :],
                                     gb_t[0:T, 0:hidden])
            if use_beta:
                nc.vector.tensor_add(o_t[0:T, :], o_t[0:T, :],
                                     gb_t[0:T, hidden:2 * hidden])
            nc.sync.dma_start(out_d[g], o_t[0:T, :])

    nc.finalize()
    return nc


def _get_program(nslot, hidden, rank, use_gamma, use_beta, variant):
    key = (nslot, hidden, rank, use_gamma, use_beta, variant)
    if key not in _PROGRAM_CACHE:
        _PROGRAM_CACHE[key] = _build_program(nslot, hidden, rank, use_gamma,
                                             use_beta, variant)
    return _PROGRAM_CACHE[key]


def kernel(h, log_alpha, ln_gamma, ln_beta, projection_u, projection_v,
           agent_ids):
    h = np.asarray(h, dtype=np.float32)
    projection_u = np.asarray(projection_u, dtype=np.float32)
    projection_v = np.asarray(projection_v, dtype=np.float32)
    ln_gamma = np.asarray(ln_gamma, dtype=np.float32)
    ln_beta = np.asarray(ln_beta, dtype=np.float32)
    ids_raw = np.asarray(agent_ids)
    log_alpha = np.float32(np.asarray(log_alpha))

    B, H = h.shape
    A, H2, R = projection_u.shape
    ids = (ids_raw.astype(np.int64) % A).astype(np.int32)

    if H != 1024 or H2 != H or R != 32 or projection_v.shape != (A, R, H):
        return _reference_numpy(h, log_alpha, ln_gamma, ln_beta, projection_u,
                                projection_v, agent_ids)

    alpha = np.float32(min(np.exp(log_alpha), np.float32(ALPHA_MAX)))
    use_gamma = not np.all(ln_gamma == 1.0)
    use_beta = not np.all(ln_beta == 0.0)

    # ---- host-side MoE dispatch: sort tokens by agent, build capacity slots
    order = np.argsort(ids, kind="stable").astype(np.int64)
    counts = np.bincount(ids, minlength=A)
    starts = np.zeros(A + 1, np.int64)
    np.cumsum(counts, out=starts[1:])

    slot_agent = []   # agent id per slot
    slot_rows = []    # (start, n) into `order` per slot
    for a in range(A):
        n = int(counts[a])
        s = int(starts[a])
        while n > 0:
            take = min(n, CAP)
            slot_agent.append(a)
            slot_rows.append((s, take))
            s += take
            n -= take
    total_slots = len(slot_agent)
    nslot = -(-total_slots // N_CORES)
    nslot = max(nslot, G)
    if nslot % G:
        nslot += G - nslot % G
    if nslot > 96:  # way off the expected distribution; play it safe
        return _reference_numpy(h, log_alpha, ln_gamma, ln_beta, projection_u,
                                projection_v, agent_ids)
    while len(slot_agent) < nslot * N_CORES:
        slot_agent.append(0)
        slot_rows.append((0, 0))
    slot_agent = np.asarray(slot_agent, np.int64)

    ngroup = nslot // G
    nchunk = H // 128
    T = G * CAP
    KR = G * R

    # row_idx: global token index feeding each padded row (clamped for pads)
    nrows = nslot * CAP
    row_idx = np.zeros((N_CORES, nrows), np.int64)
    row_valid = np.zeros((N_CORES, nrows), bool)
    for j, (s, n) in enumerate(slot_rows):
        core, sl = divmod(j, nslot)
        r0 = sl * CAP
        if n:
            row_idx[core, r0:r0 + n] = order[s:s + n]
            row_valid[core, r0:r0 + n] = True

    h_pk = np.ascontiguousarray(h[row_idx]).reshape(N_CORES, ngroup, T, H)
    # hT per group: [p(128), c(8), t(T)]
    hT_sw = np.ascontiguousarray(
        h_pk.reshape(N_CORES, ngroup, T, nchunk, 128)
        .transpose(0, 1, 4, 3, 2)).reshape(N_CORES, ngroup, 128, nchunk * T)

    sa = slot_agent.reshape(N_CORES, nslot)
    # u: [g, p(128), c(8), s(G), r(32)]
    u_sw = np.ascontiguousarray(
        projection_u[sa]                                  # [8, ns, H, R]
        .reshape(N_CORES, ngroup, G, nchunk, 128, R)
        .transpose(0, 1, 4, 3, 2, 5)                      # [8, g, 128, c, G, R]
    ).reshape(N_CORES, ngroup, 128, nchunk * KR)
    v_sw = np.ascontiguousarray(alpha * projection_v[sa]).reshape(
        N_CORES, ngroup, KR, H)

    in_maps = []
    for core in range(N_CORES):
        m = {
            "u_sw": u_sw[core],
            "v_sw": v_sw[core],
            "hT_sw": hT_sw[core],
            "h_pk": h_pk[core],
        }
        if use_gamma or use_beta:
            m["gb_rep"] = np.ascontiguousarray(
                np.stack([np.broadcast_to(ln_gamma, (128, H)),
                          np.broadcast_to(ln_beta, (128, H))]))
        in_maps.append(m)

    nc = _get_program(nslot, H, R, use_gamma, use_beta, VARIANT)

    from concourse.bass_utils import run_bass_kernel_spmd
    res = run_bass_kernel_spmd(nc, in_maps, list(range(N_CORES)))

    out = np.empty_like(h)
    for core in range(N_CORES):
        o = np.asarray(res.results[core]["out_pk"]).reshape(nrows, H)
        out[row_idx[core][row_valid[core]]] = o[row_valid[core]]
    return out


# revision 5
# speedup vs baseline: 2.3472x; 1.2553x over previous
"""Trainium2 Bass kernel for DiversityInjection (MoE-style per-agent low-rank
perturbation + LayerNorm).

Strategy: expert-parallel over the 256 agents. The host routes tokens to the
core that owns their agent (MoE dispatch done host-side), packs them into
fixed-capacity per-agent slots (CAP tokens), and each core runs dense batched
matmuls over groups of G=3 slots (126 tokens per group tile):

  mm1 (3 slots at once): psum1[96, 126] = [U_a|U_b|U_c]^T @ hT3
        8 contract chunks of 128; useful output = 3 diagonal [32, 42] blocks
  mm2 (3 slots at once, block-diag): psum2[126, 512] =
        blockdiag(intT_a, intT_b, intT_c)^T(96x126) @ [V_a; V_b; V_c](96x512)
  out = LayerNorm(h + pert) fused via bn_stats + scalar activation

The padded output is scattered back to original token order on the host.
"""

import os
import sys

for _p in ("/opt/trn_rl_repo", "/root/.axon_site/_ro/trn_rl_repo"):
    if os.path.isdir(_p) and _p not in sys.path:
        sys.path.insert(0, _p)

import numpy as np

N_CORES = 8
CAP = 42           # tokens per slot (per-agent capacity)
G = 3              # slots per group tile (G*CAP <= 128, G*rank <= 128)
ALPHA_MAX = 5.0
LN_EPS = 1e-5
VARIANT = os.environ.get("BASS_KERNEL_VARIANT", "t32")

_PROGRAM_CACHE = {}


def _reference_numpy(h, log_alpha, ln_gamma, ln_beta, projection_u, projection_v,
                     agent_ids):
    """Fallback pure-numpy implementation (used only if packing does not fit)."""
    num_agents = projection_u.shape[0]
    ids = agent_ids % num_agents
    alpha = min(np.exp(np.float32(log_alpha)), np.float32(ALPHA_MAX))
    out = np.empty_like(h)
    for a in range(num_agents):
        m = ids == a
        if not m.any():
            continue
        hb = h[m]
        pert = (hb @ projection_u[a]) @ projection_v[a]
        out[m] = hb + alpha * pert
    mean = out.mean(-1, keepdims=True, dtype=np.float64)
    var = out.var(-1, keepdims=True, dtype=np.float64)
    out = (out - mean) / np.sqrt(var + LN_EPS)
    return (out * ln_gamma + ln_beta).astype(h.dtype)


def _build_program(nslot, hidden, rank, use_gamma, use_beta, variant):
    """Build the per-core Bass program. Same program runs SPMD on all 8 cores."""
    from contextlib import ExitStack

    import concourse.bacc as bacc
    import concourse.mybir as mybir
    import concourse.tile as tile

    assert hidden == 1024 and rank == 32
    assert nslot % G == 0
    ngroup = nslot // G
    nchunk = hidden // 128
    T = G * CAP          # tokens per group tile (126)
    KR = G * rank        # stacked rank (96)

    mmdt = mybir.dt.float32r if variant.endswith("r") else mybir.dt.float32

    nc = bacc.Bacc("TRN2", target_bir_lowering=False, debug=False)

    u_d = nc.dram_tensor("u_sw", (ngroup, 128, nchunk * KR), mmdt,
                         kind="ExternalInput")
    v_d = nc.dram_tensor("v_sw", (ngroup, KR, hidden), mmdt,
                         kind="ExternalInput")
    hT_d = nc.dram_tensor("hT_sw", (ngroup, 128, nchunk * T), mmdt,
                          kind="ExternalInput")
    h_d = nc.dram_tensor("h_pk", (ngroup, T, hidden), mybir.dt.float32,
                         kind="ExternalInput")
    gb_d = None
    if use_gamma or use_beta:
        gb_d = nc.dram_tensor("gb_rep", (2, 128, hidden), mybir.dt.float32,
                              kind="ExternalInput")
    out_d = nc.dram_tensor("out_pk", (ngroup, T, hidden), mybir.dt.float32,
                           kind="ExternalOutput")

    with tile.TileContext(nc) as tc, ExitStack() as ctx:
        upool = ctx.enter_context(tc.tile_pool(name="u", bufs=4))
        vpool = ctx.enter_context(tc.tile_pool(name="v", bufs=4))
        htpool = ctx.enter_context(tc.tile_pool(name="hT", bufs=4))
        hpool = ctx.enter_context(tc.tile_pool(name="h", bufs=4))
        bpool = ctx.enter_context(tc.tile_pool(name="blk", bufs=3))
        spool = ctx.enter_context(tc.tile_pool(name="stats", bufs=8))
        xpool = ctx.enter_context(tc.tile_pool(name="x", bufs=4))
        opool = ctx.enter_context(tc.tile_pool(name="o", bufs=4))
        cpool = ctx.enter_context(tc.tile_pool(name="const", bufs=1))
        p1pool = ctx.enter_context(tc.tile_pool(name="psum1", bufs=2, space="PSUM"))
        p2pool = ctx.enter_context(tc.tile_pool(name="psum2", bufs=3, space="PSUM"))

        eps_t = cpool.tile([128, 1], mybir.dt.float32)
        nc.vector.memset(eps_t[:], LN_EPS)
        gb_t = None
        if gb_d is not None:
            gb_t = cpool.tile([128, 2 * hidden], mybir.dt.float32)
            nc.sync.dma_start(
                gb_t[:].rearrange("p (g f) -> g p f", g=2), gb_d.ap())

        for g in range(ngroup):
            u_t = upool.tile([128, nchunk * KR], mmdt)
            nc.sync.dma_start(u_t[:], u_d[g])
            hT_t = htpool.tile([128, nchunk * T], mmdt)
            nc.sync.dma_start(hT_t[:], hT_d[g])

            psum1 = p1pool.tile([KR, T], mybir.dt.float32)
            for c in range(nchunk):
                nc.tensor.matmul(
                    psum1[:],
                    u_t[:, c * KR:(c + 1) * KR],
                    hT_t[:, c * T:(c + 1) * T],
                    start=(c == 0), stop=(c == nchunk - 1),
                )

            # block-diag [KR, T] lhsT: diagonal [rank, CAP] blocks from psum1
            blk = bpool.tile([KR, T], mmdt)
            for s in range(G):
                for s2 in range(G):
                    if s == s2:
                        nc.scalar.copy(
                            blk[s * rank:(s + 1) * rank,
                                s2 * CAP:(s2 + 1) * CAP],
                            psum1[s * rank:(s + 1) * rank,
                                  s2 * CAP:(s2 + 1) * CAP])
                    else:
                        nc.gpsimd.memset(
                            blk[s * rank:(s + 1) * rank,
                                s2 * CAP:(s2 + 1) * CAP], 0.0)

            v_t = vpool.tile([KR, hidden], mmdt)
            nc.scalar.dma_start(v_t[:], v_d[g])
            h_t = hpool.tile([128, hidden], mybir.dt.float32)
            nc.scalar.dma_start(h_t[0:T, :], h_d[g])

            psum2 = p2pool.tile([128, hidden], mybir.dt.float32)
            for q in range(hidden // 512):
                nc.tensor.matmul(
                    psum2[0:T, q * 512:(q + 1) * 512],
                    blk[:],
                    v_t[:, q * 512:(q + 1) * 512],
                    start=True, stop=True,
                )

            # x = h + pert  (DVE reads PSUM + SBUF)
            x_t = xpool.tile([128, hidden], mybir.dt.float32)
            nc.vector.tensor_add(x_t[0:T, :], psum2[0:T, :], h_t[0:T, :])
            stats = spool.tile([128, 6 * (hidden // 512)], mybir.dt.float32)
            for q in range(hidden // 512):
                nc.vector.bn_stats(stats[0:T, q * 6:(q + 1) * 6],
                                   x_t[0:T, q * 512:(q + 1) * 512])
            aggr = spool.tile([128, 2], mybir.dt.float32)
            nc.vector.bn_aggr(aggr[0:T, :],
                              stats[0:T, :].rearrange("p (c s) -> p c s", s=3))
            std = spool.tile([128, 1], mybir.dt.float32)
            nc.scalar.activation(std[0:T, :], aggr[0:T, 1:2],
                                 mybir.ActivationFunctionType.Sqrt,
                                 bias=eps_t[0:T, 0:1])
            rstd = spool.tile([128, 1], mybir.dt.float32)
            nc.vector.reciprocal(rstd[0:T, :], std[0:T, :])
            nmr = spool.tile([128, 1], mybir.dt.float32)
            nc.vector.scalar_tensor_tensor(nmr[0:T, :], aggr[0:T, 0:1], -1.0,
                                           rstd[0:T, :],
                                           mybir.AluOpType.mult,
                                           mybir.AluOpType.mult)
            o_t = opool.tile([128, hidden], mybir.dt.float32)
            nc.scalar.activation(o_t[0:T, :], x_t[0:T, :],
                                 mybir.ActivationFunctionType.Identity,
                                 bias=nmr[0:T, 0:1], scale=rstd[0:T, 0:1])
            if use_gamma:
                nc.vector.tensor_mul(o_t[0:T, :], o_t[0:T, :],
                                     gb_t[0:T, 0:hidden])
            if use_beta:
                nc.vector.tensor_add(o_t[0:T, :], o_t[0:T, :],
                                     gb_t[0:T, hidden:2 * hidden])
            nc.gpsimd.dma_start(out_d[g], o_t[0:T, :])

    nc.finalize()
    return nc


def _get_program(nslot, hidden, rank, use_gamma, use_beta, variant):
    key = (nslot, hidden, rank, use_gamma, use_beta, variant)
    if key not in _PROGRAM_CACHE:
        _PROGRAM_CACHE[key] = _build_program(nslot, hidden, rank, use_gamma,
                                             use_beta, variant)
    return _PROGRAM_CACHE[key]


def kernel(h, log_alpha, ln_gamma, ln_beta, projection_u, projection_v,
           agent_ids):
    h = np.asarray(h, dtype=np.float32)
    projection_u = np.asarray(projection_u, dtype=np.float32)
    projection_v = np.asarray(projection_v, dtype=np.float32)
    ln_gamma = np.asarray(ln_gamma, dtype=np.float32)
    ln_beta = np.asarray(ln_beta, dtype=np.float32)
    ids_raw = np.asarray(agent_ids)
    log_alpha = np.float32(np.asarray(log_alpha))

    B, H = h.shape
    A, H2, R = projection_u.shape
    ids = (ids_raw.astype(np.int64) % A).astype(np.int32)

    if H != 1024 or H2 != H or R != 32 or projection_v.shape != (A, R, H):
        return _reference_numpy(h, log_alpha, ln_gamma, ln_beta, projection_u,
                                projection_v, agent_ids)

    alpha = np.float32(min(np.exp(log_alpha), np.float32(ALPHA_MAX)))
    use_gamma = not np.all(ln_gamma == 1.0)
    use_beta = not np.all(ln_beta == 0.0)

    # ---- host-side MoE dispatch: sort tokens by agent, build capacity slots
    order = np.argsort(ids, kind="stable").astype(np.int64)
    counts = np.bincount(ids, minlength=A)
    starts = np.zeros(A + 1, np.int64)
    np.cumsum(counts, out=starts[1:])

    slot_agent = []   # agent id per slot
    slot_rows = []    # (start, n) into `order` per slot
    for a in range(A):
        n = int(counts[a])
        s = int(starts[a])
        while n > 0:
            take = min(n, CAP)
            slot_agent.append(a)
            slot_rows.append((s, take))
            s += take
            n -= take
    total_slots = len(slot_agent)
    nslot = -(-total_slots // N_CORES)
    nslot = max(nslot, G)
    if nslot % G:
        nslot += G - nslot % G
    if nslot > 96:  # way off the expected distribution; play it safe
        return _reference_numpy(h, log_alpha, ln_gamma, ln_beta, projection_u,
                                projection_v, agent_ids)
    while len(slot_agent) < nslot * N_CORES:
        slot_agent.append(0)
        slot_rows.append((0, 0))
    slot_agent = np.asarray(slot_agent, np.int64)

    ngroup = nslot // G
    nchunk = H // 128
    T = G * CAP
    KR = G * R

    # row_idx: global token index feeding each padded row (clamped for pads)
    nrows = nslot * CAP
    row_idx = np.zeros((N_CORES, nrows), np.int64)
    row_valid = np.zeros((N_CORES, nrows), bool)
    for j, (s, n) in enumerate(slot_rows):
        core, sl = divmod(j, nslot)
        r0 = sl * CAP
        if n:
            row_idx[core, r0:r0 + n] = order[s:s + n]
            row_valid[core, r0:r0 + n] = True

    h_pk = np.ascontiguousarray(h[row_idx]).reshape(N_CORES, ngroup, T, H)
    # hT per group: [p(128), c(8), t(T)]
    hT_sw = np.ascontiguousarray(
        h_pk.reshape(N_CORES, ngroup, T, nchunk, 128)
        .transpose(0, 1, 4, 3, 2)).reshape(N_CORES, ngroup, 128, nchunk * T)

    sa = slot_agent.reshape(N_CORES, nslot)
    # u: [g, p(128), c(8), s(G), r(32)]
    u_sw = np.ascontiguousarray(
        projection_u[sa]                                  # [8, ns, H, R]
        .reshape(N_CORES, ngroup, G, nchunk, 128, R)
        .transpose(0, 1, 4, 3, 2, 5)                      # [8, g, 128, c, G, R]
    ).reshape(N_CORES, ngroup, 128, nchunk * KR)
    v_sw = np.ascontiguousarray(alpha * projection_v[sa]).reshape(
        N_CORES, ngroup, KR, H)

    in_maps = []
    for core in range(N_CORES):
        m = {
            "u_sw": u_sw[core],
            "v_sw": v_sw[core],
            "hT_sw": hT_sw[core],
            "h_pk": h_pk[core],
        }
        if use_gamma or use_beta:
            m["gb_rep"] = np.ascontiguousarray(
                np.stack([np.broadcast_to(ln_gamma, (128, H)),
                          np.broadcast_to(ln_beta, (128, H))]))
        in_maps.append(m)

    nc = _get_program(nslot, H, R, use_gamma, use_beta, VARIANT)

    from concourse.bass_utils import run_bass_kernel_spmd
    res = run_bass_kernel_spmd(nc, in_maps, list(range(N_CORES)))

    out = np.empty_like(h)
    for core in range(N_CORES):
        o = np.asarray(res.results[core]["out_pk"]).reshape(nrows, H)
        out[row_idx[core][row_valid[core]]] = o[row_valid[core]]
    return out


# revision 6
# speedup vs baseline: 2.3812x; 1.0145x over previous
"""Trainium2 Bass kernel for DiversityInjection (MoE-style per-agent low-rank
perturbation + LayerNorm).

Strategy: expert-parallel over the 256 agents. The host routes tokens to the
core that owns their agent (MoE dispatch done host-side), packs them into
fixed-capacity per-agent slots (CAP tokens), and each core runs dense batched
matmuls over groups of G=3 slots (126 tokens per group tile):

  mm1 (3 slots at once): psum1[96, 126] = [U_a|U_b|U_c]^T @ hT3
        8 contract chunks of 128; useful output = 3 diagonal [32, 42] blocks
  mm2 (3 slots at once, block-diag): psum2[126, 512] =
        blockdiag(intT_a, intT_b, intT_c)^T(96x126) @ [V_a; V_b; V_c](96x512)
  out = LayerNorm(h + pert) fused via bn_stats + scalar activation

The padded output is scattered back to original token order on the host.
"""

import os
import sys

for _p in ("/opt/trn_rl_repo", "/root/.axon_site/_ro/trn_rl_repo"):
    if os.path.isdir(_p) and _p not in sys.path:
        sys.path.insert(0, _p)

import numpy as np

N_CORES = 8
CAP = 42           # tokens per slot (per-agent capacity)
G = 3              # slots per group tile (G*CAP <= 128, G*rank <= 128)
ALPHA_MAX = 5.0
LN_EPS = 1e-5
VARIANT = os.environ.get("BASS_KERNEL_VARIANT", "t32")

_PROGRAM_CACHE = {}


def _reference_numpy(h, log_alpha, ln_gamma, ln_beta, projection_u, projection_v,
                     agent_ids):
    """Fallback pure-numpy implementation (used only if packing does not fit)."""
    num_agents = projection_u.shape[0]
    ids = agent_ids % num_agents
    alpha = min(np.exp(np.float32(log_alpha)), np.float32(ALPHA_MAX))
    out = np.empty_like(h)
    for a in range(num_agents):
        m = ids == a
        if not m.any():
            continue
        hb = h[m]
        pert = (hb @ projection_u[a]) @ projection_v[a]
        out[m] = hb + alpha * pert
    mean = out.mean(-1, keepdims=True, dtype=np.float64)
    var = out.var(-1, keepdims=True, dtype=np.float64)
    out = (out - mean) / np.sqrt(var + LN_EPS)
    return (out * ln_gamma + ln_beta).astype(h.dtype)


def _build_program(nslot, hidden, rank, use_gamma, use_beta, variant):
    """Build the per-core Bass program. Same program runs SPMD on all 8 cores."""
    from contextlib import ExitStack

    import concourse.bacc as bacc
    import concourse.mybir as mybir
    import concourse.tile as tile

    assert hidden == 1024 and rank == 32
    assert nslot % G == 0
    ngroup = nslot // G
    nchunk = hidden // 128
    T = G * CAP          # tokens per group tile (126)
    KR = G * rank        # stacked rank (96)

    mmdt = mybir.dt.float32r if variant.endswith("r") else mybir.dt.float32

    nc = bacc.Bacc("TRN2", target_bir_lowering=False, debug=False)

    u_d = nc.dram_tensor("u_sw", (ngroup, 128, nchunk * KR), mmdt,
                         kind="ExternalInput")
    v_d = nc.dram_tensor("v_sw", (ngroup, KR, hidden), mmdt,
                         kind="ExternalInput")
    hT_d = nc.dram_tensor("hT_sw", (ngroup, 128, nchunk * T), mmdt,
                          kind="ExternalInput")
    id_d = nc.dram_tensor("ident", (128, 128), mybir.dt.float32,
                          kind="ExternalInput")
    gb_d = None
    if use_gamma or use_beta:
        gb_d = nc.dram_tensor("gb_rep", (2, 128, hidden), mybir.dt.float32,
                              kind="ExternalInput")
    out_d = nc.dram_tensor("out_pk", (ngroup, T, hidden), mybir.dt.float32,
                           kind="ExternalOutput")

    with tile.TileContext(nc) as tc, ExitStack() as ctx:
        upool = ctx.enter_context(tc.tile_pool(name="u", bufs=4))
        vpool = ctx.enter_context(tc.tile_pool(name="v", bufs=4))
        htpool = ctx.enter_context(tc.tile_pool(name="hT", bufs=4))
        bpool = ctx.enter_context(tc.tile_pool(name="blk", bufs=3))
        spool = ctx.enter_context(tc.tile_pool(name="stats", bufs=8))
        opool = ctx.enter_context(tc.tile_pool(name="o", bufs=4))
        cpool = ctx.enter_context(tc.tile_pool(name="const", bufs=1))
        p1pool = ctx.enter_context(tc.tile_pool(name="psum1", bufs=2, space="PSUM"))
        p2pool = ctx.enter_context(tc.tile_pool(name="psum2", bufs=3, space="PSUM"))

        eps_t = cpool.tile([128, 1], mybir.dt.float32)
        nc.vector.memset(eps_t[:], LN_EPS)
        id_t = cpool.tile([128, 128], mybir.dt.float32)
        nc.sync.dma_start(id_t[:], id_d[:])
        gb_t = None
        if gb_d is not None:
            gb_t = cpool.tile([128, 2 * hidden], mybir.dt.float32)
            nc.sync.dma_start(
                gb_t[:].rearrange("p (g f) -> g p f", g=2), gb_d.ap())

        for g in range(ngroup):
            u_t = upool.tile([128, nchunk * KR], mmdt)
            nc.sync.dma_start(u_t[:], u_d[g])
            hT_t = htpool.tile([128, nchunk * T], mmdt)
            nc.sync.dma_start(hT_t[:], hT_d[g])

            psum1 = p1pool.tile([KR, T], mybir.dt.float32)
            for c in range(nchunk):
                nc.tensor.matmul(
                    psum1[:],
                    u_t[:, c * KR:(c + 1) * KR],
                    hT_t[:, c * T:(c + 1) * T],
                    start=(c == 0), stop=(c == nchunk - 1),
                )

            # block-diag [KR, T] lhsT: diagonal [rank, CAP] blocks from psum1
            blk = bpool.tile([KR, T], mmdt)
            for s in range(G):
                for s2 in range(G):
                    if s == s2:
                        nc.scalar.copy(
                            blk[s * rank:(s + 1) * rank,
                                s2 * CAP:(s2 + 1) * CAP],
                            psum1[s * rank:(s + 1) * rank,
                                  s2 * CAP:(s2 + 1) * CAP])
                    else:
                        nc.gpsimd.memset(
                            blk[s * rank:(s + 1) * rank,
                                s2 * CAP:(s2 + 1) * CAP], 0.0)

            v_t = vpool.tile([KR, hidden], mmdt)
            nc.scalar.dma_start(v_t[:], v_d[g])

            psum2 = p2pool.tile([128, hidden], mybir.dt.float32)
            for q in range(hidden // 512):
                nc.tensor.matmul(
                    psum2[0:T, q * 512:(q + 1) * 512],
                    blk[:],
                    v_t[:, q * 512:(q + 1) * 512],
                    start=True, stop=True,
                )
            # accumulate the residual h into psum2 by transposing hT chunks
            # through the PE (x = h + pert materializes in PSUM, no h reload)
            for c in range(nchunk):
                nc.tensor.matmul(
                    psum2[0:T, c * 128:(c + 1) * 128],
                    hT_t[:, c * T:(c + 1) * T],
                    id_t[:],
                    is_transpose=True, start=False, stop=True,
                    skip_group_check=True,
                )

            stats = spool.tile([128, 6 * (hidden // 512)], mybir.dt.float32)
            for q in range(hidden // 512):
                nc.vector.bn_stats(stats[0:T, q * 6:(q + 1) * 6],
                                   psum2[0:T, q * 512:(q + 1) * 512])
            aggr = spool.tile([128, 2], mybir.dt.float32)
            nc.vector.bn_aggr(aggr[0:T, :],
                              stats[0:T, :].rearrange("p (c s) -> p c s", s=3))
            std = spool.tile([128, 1], mybir.dt.float32)
            nc.scalar.activation(std[0:T, :], aggr[0:T, 1:2],
                                 mybir.ActivationFunctionType.Sqrt,
                                 bias=eps_t[0:T, 0:1])
            rstd = spool.tile([128, 1], mybir.dt.float32)
            nc.vector.reciprocal(rstd[0:T, :], std[0:T, :])
            nmr = spool.tile([128, 1], mybir.dt.float32)
            nc.vector.scalar_tensor_tensor(nmr[0:T, :], aggr[0:T, 0:1], -1.0,
                                           rstd[0:T, :],
                                           mybir.AluOpType.mult,
                                           mybir.AluOpType.mult)
            o_t = opool.tile([128, hidden], mybir.dt.float32)
            nc.scalar.activation(o_t[0:T, :], psum2[0:T, :],
                                 mybir.ActivationFunctionType.Identity,
                                 bias=nmr[0:T, 0:1], scale=rstd[0:T, 0:1])
            if use_gamma:
                nc.vector.tensor_mul(o_t[0:T, :], o_t[0:T, :],
                                     gb_t[0:T, 0:hidden])
            if use_beta:
                nc.vector.tensor_add(o_t[0:T, :], o_t[0:T, :],
                                     gb_t[0:T, hidden:2 * hidden])
            nc.scalar.dma_start(out_d[g], o_t[0:T, :])

    nc.finalize()
    return nc


def _get_program(nslot, hidden, rank, use_gamma, use_beta, variant):
    key = (nslot, hidden, rank, use_gamma, use_beta, variant)
    if key not in _PROGRAM_CACHE:
        _PROGRAM_CACHE[key] = _build_program(nslot, hidden, rank, use_gamma,
                                             use_beta, variant)
    return _PROGRAM_CACHE[key]


def kernel(h, log_alpha, ln_gamma, ln_beta, projection_u, projection_v,
           agent_ids):
    h = np.asarray(h, dtype=np.float32)
    projection_u = np.asarray(projection_u, dtype=np.float32)
    projection_v = np.asarray(projection_v, dtype=np.float32)
    ln_gamma = np.asarray(ln_gamma, dtype=np.float32)
    ln_beta = np.asarray(ln_beta, dtype=np.float32)
    ids_raw = np.asarray(agent_ids)
    log_alpha = np.float32(np.asarray(log_alpha))

    B, H = h.shape
    A, H2, R = projection_u.shape
    ids = (ids_raw.astype(np.int64) % A).astype(np.int32)

    if H != 1024 or H2 != H or R != 32 or projection_v.shape != (A, R, H):
        return _reference_numpy(h, log_alpha, ln_gamma, ln_beta, projection_u,
                                projection_v, agent_ids)

    alpha = np.float32(min(np.exp(log_alpha), np.float32(ALPHA_MAX)))
    use_gamma = not np.all(ln_gamma == 1.0)
    use_beta = not np.all(ln_beta == 0.0)

    # ---- host-side MoE dispatch: sort tokens by agent, build capacity slots
    order = np.argsort(ids, kind="stable").astype(np.int64)
    counts = np.bincount(ids, minlength=A)
    starts = np.zeros(A + 1, np.int64)
    np.cumsum(counts, out=starts[1:])

    slot_agent = []   # agent id per slot
    slot_rows = []    # (start, n) into `order` per slot
    for a in range(A):
        n = int(counts[a])
        s = int(starts[a])
        while n > 0:
            take = min(n, CAP)
            slot_agent.append(a)
            slot_rows.append((s, take))
            s += take
            n -= take
    total_slots = len(slot_agent)
    nslot = -(-total_slots // N_CORES)
    nslot = max(nslot, G)
    if nslot % G:
        nslot += G - nslot % G
    if nslot > 96:  # way off the expected distribution; play it safe
        return _reference_numpy(h, log_alpha, ln_gamma, ln_beta, projection_u,
                                projection_v, agent_ids)
    while len(slot_agent) < nslot * N_CORES:
        slot_agent.append(0)
        slot_rows.append((0, 0))
    slot_agent = np.asarray(slot_agent, np.int64)

    ngroup = nslot // G
    nchunk = H // 128
    T = G * CAP
    KR = G * R

    # row_idx: global token index feeding each padded row (clamped for pads)
    nrows = nslot * CAP
    row_idx = np.zeros((N_CORES, nrows), np.int64)
    row_valid = np.zeros((N_CORES, nrows), bool)
    for j, (s, n) in enumerate(slot_rows):
        core, sl = divmod(j, nslot)
        r0 = sl * CAP
        if n:
            row_idx[core, r0:r0 + n] = order[s:s + n]
            row_valid[core, r0:r0 + n] = True

    h_pk = h[row_idx].reshape(N_CORES, ngroup, T, H)
    # hT per group: [p(128), c(8), t(T)]
    hT_sw = np.ascontiguousarray(
        h_pk.reshape(N_CORES, ngroup, T, nchunk, 128)
        .transpose(0, 1, 4, 3, 2)).reshape(N_CORES, ngroup, 128, nchunk * T)
    ident = np.eye(128, dtype=np.float32)

    sa = slot_agent.reshape(N_CORES, nslot)
    # u: [g, p(128), c(8), s(G), r(32)]
    u_sw = np.ascontiguousarray(
        projection_u[sa]                                  # [8, ns, H, R]
        .reshape(N_CORES, ngroup, G, nchunk, 128, R)
        .transpose(0, 1, 4, 3, 2, 5)                      # [8, g, 128, c, G, R]
    ).reshape(N_CORES, ngroup, 128, nchunk * KR)
    v_sw = np.ascontiguousarray(alpha * projection_v[sa]).reshape(
        N_CORES, ngroup, KR, H)

    in_maps = []
    for core in range(N_CORES):
        m = {
            "u_sw": u_sw[core],
            "v_sw": v_sw[core],
            "hT_sw": hT_sw[core],
            "ident": ident,
        }
        if use_gamma or use_beta:
            m["gb_rep"] = np.ascontiguousarray(
                np.stack([np.broadcast_to(ln_gamma, (128, H)),
                          np.broadcast_to(ln_beta, (128, H))]))
        in_maps.append(m)

    nc = _get_program(nslot, H, R, use_gamma, use_beta, VARIANT)

    from concourse.bass_utils import run_bass_kernel_spmd
    res = run_bass_kernel_spmd(nc, in_maps, list(range(N_CORES)))

    out = np.empty_like(h)
    for core in range(N_CORES):
        o = np.asarray(res.results[core]["out_pk"]).reshape(nrows, H)
        out[row_idx[core][row_valid[core]]] = o[row_valid[core]]
    return out


# revision 7
# speedup vs baseline: 2.7777x; 1.1665x over previous
"""Trainium2 Bass kernel for DiversityInjection (MoE-style per-agent low-rank
perturbation + LayerNorm).

Strategy: expert-parallel over the 256 agents. The host routes tokens to the
core that owns their agent (MoE dispatch done host-side), packs them into
fixed-capacity per-agent slots (CAP tokens), and each core runs dense batched
matmuls over groups of G=3 slots (126 tokens per group tile):

  mm1 (3 slots at once): psum1[96, 126] = [U_a|U_b|U_c]^T @ hT3
        8 contract chunks of 128; useful output = 3 diagonal [32, 42] blocks
  mm2 (3 slots at once, block-diag): psum2[126, 512] =
        blockdiag(intT_a, intT_b, intT_c)^T(96x126) @ [V_a; V_b; V_c](96x512)
  out = LayerNorm(h + pert) fused via bn_stats + scalar activation

The padded output is scattered back to original token order on the host.
"""

import os
import sys

for _p in ("/opt/trn_rl_repo", "/root/.axon_site/_ro/trn_rl_repo"):
    if os.path.isdir(_p) and _p not in sys.path:
        sys.path.insert(0, _p)

import numpy as np

N_CORES = 8
CAP = 42           # tokens per slot (per-agent capacity)
G = 3              # slots per group tile (G*CAP <= 128, G*rank <= 128)
ALPHA_MAX = 5.0
LN_EPS = 1e-5
VARIANT = os.environ.get("BASS_KERNEL_VARIANT", "t32")

_PROGRAM_CACHE = {}


def _reference_numpy(h, log_alpha, ln_gamma, ln_beta, projection_u, projection_v,
                     agent_ids):
    """Fallback pure-numpy implementation (used only if packing does not fit)."""
    num_agents = projection_u.shape[0]
    ids = agent_ids % num_agents
    alpha = min(np.exp(np.float32(log_alpha)), np.float32(ALPHA_MAX))
    out = np.empty_like(h)
    for a in range(num_agents):
        m = ids == a
        if not m.any():
            continue
        hb = h[m]
        pert = (hb @ projection_u[a]) @ projection_v[a]
        out[m] = hb + alpha * pert
    mean = out.mean(-1, keepdims=True, dtype=np.float64)
    var = out.var(-1, keepdims=True, dtype=np.float64)
    out = (out - mean) / np.sqrt(var + LN_EPS)
    return (out * ln_gamma + ln_beta).astype(h.dtype)


def _build_program(nslot, hidden, rank, use_gamma, use_beta, variant):
    """Build the per-core Bass program. Same program runs SPMD on all 8 cores."""
    from contextlib import ExitStack

    import concourse.bacc as bacc
    import concourse.mybir as mybir
    import concourse.tile as tile

    assert hidden == 1024 and rank == 32
    assert nslot % G == 0
    ngroup = nslot // G
    nchunk = hidden // 128
    T = G * CAP          # tokens per group tile (126)
    KR = G * rank        # stacked rank (96)

    mmdt = mybir.dt.float32r if variant.endswith("r") else mybir.dt.float32

    nc = bacc.Bacc("TRN2", target_bir_lowering=False, debug=False)

    u_d = nc.dram_tensor("u_sw", (ngroup, 128, nchunk * KR), mmdt,
                         kind="ExternalInput")
    v_d = nc.dram_tensor("v_sw", (ngroup, KR, hidden), mmdt,
                         kind="ExternalInput")
    hT_d = nc.dram_tensor("hT_sw", (ngroup, 128, nchunk * T), mmdt,
                          kind="ExternalInput")
    id_d = nc.dram_tensor("ident", (128, 128), mybir.dt.float32,
                          kind="ExternalInput")
    gb_d = None
    if use_gamma or use_beta:
        gb_d = nc.dram_tensor("gb_rep", (2, 128, hidden), mybir.dt.float32,
                              kind="ExternalInput")
    out_d = nc.dram_tensor("out_pk", (ngroup, T, hidden), mybir.dt.float32,
                           kind="ExternalOutput")

    with tile.TileContext(nc) as tc, ExitStack() as ctx:
        upool = ctx.enter_context(tc.tile_pool(name="u", bufs=4))
        vpool = ctx.enter_context(tc.tile_pool(name="v", bufs=4))
        htpool = ctx.enter_context(tc.tile_pool(name="hT", bufs=4))
        bpool = ctx.enter_context(tc.tile_pool(name="blk", bufs=3))
        spool = ctx.enter_context(tc.tile_pool(name="stats", bufs=8))
        opool = ctx.enter_context(tc.tile_pool(name="o", bufs=4))
        cpool = ctx.enter_context(tc.tile_pool(name="const", bufs=1))
        p1pool = ctx.enter_context(tc.tile_pool(name="psum1", bufs=4, space="PSUM"))
        p2pool = ctx.enter_context(tc.tile_pool(name="psum2", bufs=2, space="PSUM"))

        eps_t = cpool.tile([128, 1], mybir.dt.float32)
        nc.vector.memset(eps_t[:], LN_EPS)
        id_t = cpool.tile([128, 128], mybir.dt.float32)
        nc.gpsimd.dma_start(id_t[:], id_d[:])
        gb_t = None
        if gb_d is not None:
            gb_t = cpool.tile([128, 2 * hidden], mybir.dt.float32)
            nc.sync.dma_start(
                gb_t[:].rearrange("p (g f) -> g p f", g=2), gb_d.ap())

        for g in range(ngroup):
            u_t = upool.tile([128, nchunk * KR], mmdt)
            nc.scalar.dma_start(u_t[:], u_d[g])
            hT_t = htpool.tile([128, nchunk * T], mmdt)
            nc.sync.dma_start(hT_t[:], hT_d[g])

            psum1 = p1pool.tile([KR, T], mybir.dt.float32)
            for c in range(nchunk):
                nc.tensor.matmul(
                    psum1[:],
                    u_t[:, c * KR:(c + 1) * KR],
                    hT_t[:, c * T:(c + 1) * T],
                    start=(c == 0), stop=(c == nchunk - 1),
                )

            # block-diag [KR, T] lhsT: diagonal [rank, CAP] blocks from psum1
            blk = bpool.tile([KR, T], mmdt)
            for s in range(G):
                for s2 in range(G):
                    if s == s2:
                        nc.vector.tensor_copy(
                            blk[s * rank:(s + 1) * rank,
                                s2 * CAP:(s2 + 1) * CAP],
                            psum1[s * rank:(s + 1) * rank,
                                  s2 * CAP:(s2 + 1) * CAP])
                    else:
                        nc.gpsimd.memset(
                            blk[s * rank:(s + 1) * rank,
                                s2 * CAP:(s2 + 1) * CAP], 0.0)

            v_t = vpool.tile([KR, hidden], mmdt)
            nc.sync.dma_start(v_t[:], v_d[g])

            psum2 = p2pool.tile([128, hidden], mybir.dt.float32)
            for q in range(hidden // 512):
                nc.tensor.matmul(
                    psum2[0:T, q * 512:(q + 1) * 512],
                    blk[:],
                    v_t[:, q * 512:(q + 1) * 512],
                    start=True, stop=True,
                )
            # accumulate the residual h into psum2 by transposing hT chunks
            # through the PE (x = h + pert materializes in PSUM, no h reload)
            for c in range(nchunk):
                nc.tensor.matmul(
                    psum2[0:T, c * 128:(c + 1) * 128],
                    hT_t[:, c * T:(c + 1) * T],
                    id_t[:],
                    is_transpose=True, start=False, stop=True,
                    skip_group_check=True,
                )

            stats = spool.tile([128, 6 * (hidden // 512)], mybir.dt.float32)
            for q in range(hidden // 512):
                nc.vector.bn_stats(stats[0:T, q * 6:(q + 1) * 6],
                                   psum2[0:T, q * 512:(q + 1) * 512])
            aggr = spool.tile([128, 2], mybir.dt.float32)
            nc.vector.bn_aggr(aggr[0:T, :],
                              stats[0:T, :].rearrange("p (c s) -> p c s", s=3))
            std = spool.tile([128, 1], mybir.dt.float32)
            nc.scalar.activation(std[0:T, :], aggr[0:T, 1:2],
                                 mybir.ActivationFunctionType.Sqrt,
                                 bias=eps_t[0:T, 0:1])
            rstd = spool.tile([128, 1], mybir.dt.float32)
            nc.vector.reciprocal(rstd[0:T, :], std[0:T, :])
            nmr = spool.tile([128, 1], mybir.dt.float32)
            nc.vector.scalar_tensor_tensor(nmr[0:T, :], aggr[0:T, 0:1], -1.0,
                                           rstd[0:T, :],
                                           mybir.AluOpType.mult,
                                           mybir.AluOpType.mult)
            o_t = opool.tile([128, hidden], mybir.dt.float32)
            nc.scalar.activation(o_t[0:T, :], psum2[0:T, :],
                                 mybir.ActivationFunctionType.Identity,
                                 bias=nmr[0:T, 0:1], scale=rstd[0:T, 0:1])
            if use_gamma:
                nc.vector.tensor_mul(o_t[0:T, :], o_t[0:T, :],
                                     gb_t[0:T, 0:hidden])
            if use_beta:
                nc.vector.tensor_add(o_t[0:T, :], o_t[0:T, :],
                                     gb_t[0:T, hidden:2 * hidden])
            nc.gpsimd.dma_start(out_d[g], o_t[0:T, :])

    nc.finalize()
    return nc


def _get_program(nslot, hidden, rank, use_gamma, use_beta, variant):
    key = (nslot, hidden, rank, use_gamma, use_beta, variant)
    if key not in _PROGRAM_CACHE:
        _PROGRAM_CACHE[key] = _build_program(nslot, hidden, rank, use_gamma,
                                             use_beta, variant)
    return _PROGRAM_CACHE[key]


def kernel(h, log_alpha, ln_gamma, ln_beta, projection_u, projection_v,
           agent_ids):
    h = np.asarray(h, dtype=np.float32)
    projection_u = np.asarray(projection_u, dtype=np.float32)
    projection_v = np.asarray(projection_v, dtype=np.float32)
    ln_gamma = np.asarray(ln_gamma, dtype=np.float32)
    ln_beta = np.asarray(ln_beta, dtype=np.float32)
    ids_raw = np.asarray(agent_ids)
    log_alpha = np.float32(np.asarray(log_alpha))

    B, H = h.shape
    A, H2, R = projection_u.shape
    ids = (ids_raw.astype(np.int64) % A).astype(np.int32)

    if H != 1024 or H2 != H or R != 32 or projection_v.shape != (A, R, H):
        return _reference_numpy(h, log_alpha, ln_gamma, ln_beta, projection_u,
                                projection_v, agent_ids)

    alpha = np.float32(min(np.exp(log_alpha), np.float32(ALPHA_MAX)))
    use_gamma = not np.all(ln_gamma == 1.0)
    use_beta = not np.all(ln_beta == 0.0)

    # ---- host-side MoE dispatch: sort tokens by agent, build capacity slots
    order = np.argsort(ids, kind="stable").astype(np.int64)
    counts = np.bincount(ids, minlength=A)
    starts = np.zeros(A + 1, np.int64)
    np.cumsum(counts, out=starts[1:])

    slot_agent = []   # agent id per slot
    slot_rows = []    # (start, n) into `order` per slot
    for a in range(A):
        n = int(counts[a])
        s = int(starts[a])
        while n > 0:
            take = min(n, CAP)
            slot_agent.append(a)
            slot_rows.append((s, take))
            s += take
            n -= take
    total_slots = len(slot_agent)
    nslot = -(-total_slots // N_CORES)
    nslot = max(nslot, G)
    if nslot % G:
        nslot += G - nslot % G
    if nslot > 96:  # way off the expected distribution; play it safe
        return _reference_numpy(h, log_alpha, ln_gamma, ln_beta, projection_u,
                                projection_v, agent_ids)
    while len(slot_agent) < nslot * N_CORES:
        slot_agent.append(0)
        slot_rows.append((0, 0))
    slot_agent = np.asarray(slot_agent, np.int64)

    ngroup = nslot // G
    nchunk = H // 128
    T = G * CAP
    KR = G * R

    # row_idx: global token index feeding each padded row (clamped for pads)
    nrows = nslot * CAP
    row_idx = np.zeros((N_CORES, nrows), np.int64)
    row_valid = np.zeros((N_CORES, nrows), bool)
    for j, (s, n) in enumerate(slot_rows):
        core, sl = divmod(j, nslot)
        r0 = sl * CAP
        if n:
            row_idx[core, r0:r0 + n] = order[s:s + n]
            row_valid[core, r0:r0 + n] = True

    h_pk = h[row_idx].reshape(N_CORES, ngroup, T, H)
    # hT per group: [p(128), c(8), t(T)]
    hT_sw = np.ascontiguousarray(
        h_pk.reshape(N_CORES, ngroup, T, nchunk, 128)
        .transpose(0, 1, 4, 3, 2)).reshape(N_CORES, ngroup, 128, nchunk * T)
    ident = np.eye(128, dtype=np.float32)

    sa = slot_agent.reshape(N_CORES, nslot)
    # u: [g, p(128), c(8), s(G), r(32)]
    u_sw = np.ascontiguousarray(
        projection_u[sa]                                  # [8, ns, H, R]
        .reshape(N_CORES, ngroup, G, nchunk, 128, R)
        .transpose(0, 1, 4, 3, 2, 5)                      # [8, g, 128, c, G, R]
    ).reshape(N_CORES, ngroup, 128, nchunk * KR)
    v_sw = np.ascontiguousarray(alpha * projection_v[sa]).reshape(
        N_CORES, ngroup, KR, H)

    in_maps = []
    for core in range(N_CORES):
        m = {
            "u_sw": u_sw[core],
            "v_sw": v_sw[core],
            "hT_sw": hT_sw[core],
            "ident": ident,
        }
        if use_gamma or use_beta:
            m["gb_rep"] = np.ascontiguousarray(
                np.stack([np.broadcast_to(ln_gamma, (128, H)),
                          np.broadcast_to(ln_beta, (128, H))]))
        in_maps.append(m)

    nc = _get_program(nslot, H, R, use_gamma, use_beta, VARIANT)

    from concourse.bass_utils import run_bass_kernel_spmd
    res = run_bass_kernel_spmd(nc, in_maps, list(range(N_CORES)))

    out = np.empty_like(h)
    for core in range(N_CORES):
        o = np.asarray(res.results[core]["out_pk"]).reshape(nrows, H)
        out[row_idx[core][row_valid[core]]] = o[row_valid[core]]
    return out


# revision 8
# speedup vs baseline: 2.8439x; 1.0238x over previous
"""Trainium2 Bass kernel for DiversityInjection (MoE-style per-agent low-rank
perturbation + LayerNorm).

Strategy: expert-parallel over the 256 agents. The host routes tokens to the
core that owns their agent (MoE dispatch done host-side), packs them into
fixed-capacity per-agent slots (CAP tokens), and each core runs dense batched
matmuls over groups of G=3 slots (126 tokens per group tile):

  mm1 (3 slots at once): psum1[96, 126] = [U_a|U_b|U_c]^T @ hT3
        8 contract chunks of 128; useful output = 3 diagonal [32, 42] blocks
  mm2 (3 slots at once, block-diag): psum2[126, 512] =
        blockdiag(intT_a, intT_b, intT_c)^T(96x126) @ [V_a; V_b; V_c](96x512)
  out = LayerNorm(h + pert) fused via bn_stats + scalar activation

The padded output is scattered back to original token order on the host.
"""

import os
import sys

for _p in ("/opt/trn_rl_repo", "/root/.axon_site/_ro/trn_rl_repo"):
    if os.path.isdir(_p) and _p not in sys.path:
        sys.path.insert(0, _p)

import numpy as np

N_CORES = 8
CAP = 42           # tokens per slot (per-agent capacity)
G = 3              # slots per group tile (G*CAP <= 128, G*rank <= 128)
ALPHA_MAX = 5.0
LN_EPS = 1e-5
VARIANT = os.environ.get("BASS_KERNEL_VARIANT", "t32")

_PROGRAM_CACHE = {}


def _reference_numpy(h, log_alpha, ln_gamma, ln_beta, projection_u, projection_v,
                     agent_ids):
    """Fallback pure-numpy implementation (used only if packing does not fit)."""
    num_agents = projection_u.shape[0]
    ids = agent_ids % num_agents
    alpha = min(np.exp(np.float32(log_alpha)), np.float32(ALPHA_MAX))
    out = np.empty_like(h)
    for a in range(num_agents):
        m = ids == a
        if not m.any():
            continue
        hb = h[m]
        pert = (hb @ projection_u[a]) @ projection_v[a]
        out[m] = hb + alpha * pert
    mean = out.mean(-1, keepdims=True, dtype=np.float64)
    var = out.var(-1, keepdims=True, dtype=np.float64)
    out = (out - mean) / np.sqrt(var + LN_EPS)
    return (out * ln_gamma + ln_beta).astype(h.dtype)


def _build_program(nslot, hidden, rank, use_gamma, use_beta, variant):
    """Build the per-core Bass program. Same program runs SPMD on all 8 cores."""
    from contextlib import ExitStack

    import concourse.bacc as bacc
    import concourse.mybir as mybir
    import concourse.tile as tile

    assert hidden == 1024 and rank == 32
    assert nslot % G == 0
    ngroup = nslot // G
    nchunk = hidden // 128
    T = G * CAP          # tokens per group tile (126)
    KR = G * rank        # stacked rank (96)

    mmdt = mybir.dt.float32r if variant.endswith("r") else mybir.dt.float32

    nc = bacc.Bacc("TRN2", target_bir_lowering=False, debug=False)

    u_d = nc.dram_tensor("u_sw", (ngroup, 128, nchunk * KR), mmdt,
                         kind="ExternalInput")
    v_d = nc.dram_tensor("v_sw", (ngroup, KR, hidden), mmdt,
                         kind="ExternalInput")
    hT_d = nc.dram_tensor("hT_sw", (ngroup, 128, nchunk * T), mmdt,
                          kind="ExternalInput")
    id_d = nc.dram_tensor("ident", (128, 128), mybir.dt.float32,
                          kind="ExternalInput")
    gb_d = None
    if use_gamma or use_beta:
        gb_d = nc.dram_tensor("gb_rep", (2, 128, hidden), mybir.dt.float32,
                              kind="ExternalInput")
    out_d = nc.dram_tensor("out_pk", (ngroup, T, hidden), mybir.dt.float32,
                           kind="ExternalOutput")

    with tile.TileContext(nc) as tc, ExitStack() as ctx:
        upool = ctx.enter_context(tc.tile_pool(name="u", bufs=6))
        vpool = ctx.enter_context(tc.tile_pool(name="v", bufs=6))
        htpool = ctx.enter_context(tc.tile_pool(name="hT", bufs=6))
        bpool = ctx.enter_context(tc.tile_pool(name="blk", bufs=3))
        spool = ctx.enter_context(tc.tile_pool(name="stats", bufs=8))
        opool = ctx.enter_context(tc.tile_pool(name="o", bufs=6))
        cpool = ctx.enter_context(tc.tile_pool(name="const", bufs=1))
        p1pool = ctx.enter_context(tc.tile_pool(name="psum1", bufs=4, space="PSUM"))
        p2pool = ctx.enter_context(tc.tile_pool(name="psum2", bufs=2, space="PSUM"))

        eps_t = cpool.tile([128, 1], mybir.dt.float32)
        nc.vector.memset(eps_t[:], LN_EPS)
        id_t = cpool.tile([128, 128], mybir.dt.float32)
        nc.gpsimd.dma_start(id_t[:], id_d[:])
        gb_t = None
        if gb_d is not None:
            gb_t = cpool.tile([128, 2 * hidden], mybir.dt.float32)
            nc.sync.dma_start(
                gb_t[:].rearrange("p (g f) -> g p f", g=2), gb_d.ap())

        for g in range(ngroup):
            u_t = upool.tile([128, nchunk * KR], mmdt)
            hc = nchunk // 2
            nc.scalar.dma_start(u_t[:, 0:hc * KR], u_d[g][:, 0:hc * KR])
            nc.scalar.dma_start(u_t[:, hc * KR:], u_d[g][:, hc * KR:])
            hT_t = htpool.tile([128, nchunk * T], mmdt)
            nc.sync.dma_start(hT_t[:, 0:hc * T], hT_d[g][:, 0:hc * T])
            nc.sync.dma_start(hT_t[:, hc * T:], hT_d[g][:, hc * T:])

            psum1 = p1pool.tile([KR, T], mybir.dt.float32)
            for c in range(nchunk):
                nc.tensor.matmul(
                    psum1[:],
                    u_t[:, c * KR:(c + 1) * KR],
                    hT_t[:, c * T:(c + 1) * T],
                    start=(c == 0), stop=(c == nchunk - 1),
                )

            # block-diag [KR, T] lhsT: diagonal [rank, CAP] blocks from psum1
            blk = bpool.tile([KR, T], mmdt)
            for s in range(G):
                for s2 in range(G):
                    if s == s2:
                        nc.vector.tensor_copy(
                            blk[s * rank:(s + 1) * rank,
                                s2 * CAP:(s2 + 1) * CAP],
                            psum1[s * rank:(s + 1) * rank,
                                  s2 * CAP:(s2 + 1) * CAP])
                    else:
                        nc.gpsimd.memset(
                            blk[s * rank:(s + 1) * rank,
                                s2 * CAP:(s2 + 1) * CAP], 0.0)

            v_t = vpool.tile([KR, hidden], mmdt)
            nc.sync.dma_start(v_t[:], v_d[g])

            psum2 = p2pool.tile([128, hidden], mybir.dt.float32)
            for q in range(hidden // 512):
                nc.tensor.matmul(
                    psum2[0:T, q * 512:(q + 1) * 512],
                    blk[:],
                    v_t[:, q * 512:(q + 1) * 512],
                    start=True, stop=True,
                )
            # accumulate the residual h into psum2 by transposing hT chunks
            # through the PE (x = h + pert materializes in PSUM, no h reload)
            for c in range(nchunk):
                nc.tensor.matmul(
                    psum2[0:T, c * 128:(c + 1) * 128],
                    hT_t[:, c * T:(c + 1) * T],
                    id_t[:],
                    is_transpose=True, start=False, stop=True,
                    skip_group_check=True,
                )

            stats = spool.tile([128, 6 * (hidden // 512)], mybir.dt.float32)
            for q in range(hidden // 512):
                nc.vector.bn_stats(stats[0:T, q * 6:(q + 1) * 6],
                                   psum2[0:T, q * 512:(q + 1) * 512])
            aggr = spool.tile([128, 2], mybir.dt.float32)
            nc.vector.bn_aggr(aggr[0:T, :],
                              stats[0:T, :].rearrange("p (c s) -> p c s", s=3))
            std = spool.tile([128, 1], mybir.dt.float32)
            nc.scalar.activation(std[0:T, :], aggr[0:T, 1:2],
                                 mybir.ActivationFunctionType.Sqrt,
                                 bias=eps_t[0:T, 0:1])
            rstd = spool.tile([128, 1], mybir.dt.float32)
            nc.vector.reciprocal(rstd[0:T, :], std[0:T, :])
            nmr = spool.tile([128, 1], mybir.dt.float32)
            nc.vector.scalar_tensor_tensor(nmr[0:T, :], aggr[0:T, 0:1], -1.0,
                                           rstd[0:T, :],
                                           mybir.AluOpType.mult,
                                           mybir.AluOpType.mult)
            o_t = opool.tile([128, hidden], mybir.dt.float32)
            nc.scalar.activation(o_t[0:T, :], psum2[0:T, :],
                                 mybir.ActivationFunctionType.Identity,
                                 bias=nmr[0:T, 0:1], scale=rstd[0:T, 0:1])
            if use_gamma:
                nc.vector.tensor_mul(o_t[0:T, :], o_t[0:T, :],
                                     gb_t[0:T, 0:hidden])
            if use_beta:
                nc.vector.tensor_add(o_t[0:T, :], o_t[0:T, :],
                                     gb_t[0:T, hidden:2 * hidden])
            if g % 2 == 0:
                nc.gpsimd.dma_start(out_d[g], o_t[0:T, :])
            else:
                nc.sync.dma_start(out_d[g], o_t[0:T, :])

    nc.finalize()
    return nc


def _get_program(nslot, hidden, rank, use_gamma, use_beta, variant):
    key = (nslot, hidden, rank, use_gamma, use_beta, variant)
    if key not in _PROGRAM_CACHE:
        _PROGRAM_CACHE[key] = _build_program(nslot, hidden, rank, use_gamma,
                                             use_beta, variant)
    return _PROGRAM_CACHE[key]


def kernel(h, log_alpha, ln_gamma, ln_beta, projection_u, projection_v,
           agent_ids):
    h = np.asarray(h, dtype=np.float32)
    projection_u = np.asarray(projection_u, dtype=np.float32)
    projection_v = np.asarray(projection_v, dtype=np.float32)
    ln_gamma = np.asarray(ln_gamma, dtype=np.float32)
    ln_beta = np.asarray(ln_beta, dtype=np.float32)
    ids_raw = np.asarray(agent_ids)
    log_alpha = np.float32(np.asarray(log_alpha))

    B, H = h.shape
    A, H2, R = projection_u.shape
    ids = (ids_raw.astype(np.int64) % A).astype(np.int32)

    if H != 1024 or H2 != H or R != 32 or projection_v.shape != (A, R, H):
        return _reference_numpy(h, log_alpha, ln_gamma, ln_beta, projection_u,
                                projection_v, agent_ids)

    alpha = np.float32(min(np.exp(log_alpha), np.float32(ALPHA_MAX)))
    use_gamma = not np.all(ln_gamma == 1.0)
    use_beta = not np.all(ln_beta == 0.0)

    # ---- host-side MoE dispatch: sort tokens by agent, build capacity slots
    order = np.argsort(ids, kind="stable").astype(np.int64)
    counts = np.bincount(ids, minlength=A)
    starts = np.zeros(A + 1, np.int64)
    np.cumsum(counts, out=starts[1:])

    slot_agent = []   # agent id per slot
    slot_rows = []    # (start, n) into `order` per slot
    for a in range(A):
        n = int(counts[a])
        s = int(starts[a])
        while n > 0:
            take = min(n, CAP)
            slot_agent.append(a)
            slot_rows.append((s, take))
            s += take
            n -= take
    total_slots = len(slot_agent)
    nslot = -(-total_slots // N_CORES)
    nslot = max(nslot, G)
    if nslot % G:
        nslot += G - nslot % G
    if nslot > 96:  # way off the expected distribution; play it safe
        return _reference_numpy(h, log_alpha, ln_gamma, ln_beta, projection_u,
                                projection_v, agent_ids)
    while len(slot_agent) < nslot * N_CORES:
        slot_agent.append(0)
        slot_rows.append((0, 0))
    slot_agent = np.asarray(slot_agent, np.int64)

    ngroup = nslot // G
    nchunk = H // 128
    T = G * CAP
    KR = G * R

    # row_idx: global token index feeding each padded row (clamped for pads)
    nrows = nslot * CAP
    row_idx = np.zeros((N_CORES, nrows), np.int64)
    row_valid = np.zeros((N_CORES, nrows), bool)
    for j, (s, n) in enumerate(slot_rows):
        core, sl = divmod(j, nslot)
        r0 = sl * CAP
        if n:
            row_idx[core, r0:r0 + n] = order[s:s + n]
            row_valid[core, r0:r0 + n] = True

    h_pk = h[row_idx].reshape(N_CORES, ngroup, T, H)
    # hT per group: [p(128), c(8), t(T)]
    hT_sw = np.ascontiguousarray(
        h_pk.reshape(N_CORES, ngroup, T, nchunk, 128)
        .transpose(0, 1, 4, 3, 2)).reshape(N_CORES, ngroup, 128, nchunk * T)
    ident = np.eye(128, dtype=np.float32)

    sa = slot_agent.reshape(N_CORES, nslot)
    # u: [g, p(128), c(8), s(G), r(32)]
    u_sw = np.ascontiguousarray(
        projection_u[sa]                                  # [8, ns, H, R]
        .reshape(N_CORES, ngroup, G, nchunk, 128, R)
        .transpose(0, 1, 4, 3, 2, 5)                      # [8, g, 128, c, G, R]
    ).reshape(N_CORES, ngroup, 128, nchunk * KR)
    v_sw = np.ascontiguousarray(alpha * projection_v[sa]).reshape(
        N_CORES, ngroup, KR, H)

    in_maps = []
    for core in range(N_CORES):
        m = {
            "u_sw": u_sw[core],
            "v_sw": v_sw[core],
            "hT_sw": hT_sw[core],
            "ident": ident,
        }
        if use_gamma or use_beta:
            m["gb_rep"] = np.ascontiguousarray(
                np.stack([np.broadcast_to(ln_gamma, (128, H)),
                          np.broadcast_to(ln_beta, (128, H))]))
        in_maps.append(m)

    nc = _get_program(nslot, H, R, use_gamma, use_beta, VARIANT)

    from concourse.bass_utils import run_bass_kernel_spmd
    res = run_bass_kernel_spmd(nc, in_maps, list(range(N_CORES)))

    out = np.empty_like(h)
    for core in range(N_CORES):
        o = np.asarray(res.results[core]["out_pk"]).reshape(nrows, H)
        out[row_idx[core][row_valid[core]]] = o[row_valid[core]]
    return out


# revision 9
# speedup vs baseline: 2.9617x; 1.0414x over previous
"""Trainium2 Bass kernel for DiversityInjection (MoE-style per-agent low-rank
perturbation + LayerNorm).

Strategy: expert-parallel over the 256 agents. The host routes tokens to the
core that owns their agent (MoE dispatch done host-side), packs them into
fixed-capacity per-agent slots (CAP tokens), and each core runs dense batched
matmuls over groups of G=3 slots (126 tokens per group tile):

  mm1 (3 slots at once): psum1[96, 126] = [U_a|U_b|U_c]^T @ hT3
        8 contract chunks of 128; useful output = 3 diagonal [32, 42] blocks
  mm2 (3 slots at once, block-diag): psum2[126, 512] =
        blockdiag(intT_a, intT_b, intT_c)^T(96x126) @ [V_a; V_b; V_c](96x512)
  out = LayerNorm(h + pert) fused via bn_stats + scalar activation

The padded output is scattered back to original token order on the host.
"""

import os
import sys

for _p in ("/opt/trn_rl_repo", "/root/.axon_site/_ro/trn_rl_repo"):
    if os.path.isdir(_p) and _p not in sys.path:
        sys.path.insert(0, _p)

import numpy as np

N_CORES = 8
CAP = 42           # tokens per slot (per-agent capacity)
G = 3              # slots per group tile (G*CAP <= 128, G*rank <= 128)
ALPHA_MAX = 5.0
LN_EPS = 1e-5
VARIANT = os.environ.get("BASS_KERNEL_VARIANT", "t32")

_PROGRAM_CACHE = {}


def _reference_numpy(h, log_alpha, ln_gamma, ln_beta, projection_u, projection_v,
                     agent_ids):
    """Fallback pure-numpy implementation (used only if packing does not fit)."""
    num_agents = projection_u.shape[0]
    ids = agent_ids % num_agents
    alpha = min(np.exp(np.float32(log_alpha)), np.float32(ALPHA_MAX))
    out = np.empty_like(h)
    for a in range(num_agents):
        m = ids == a
        if not m.any():
            continue
        hb = h[m]
        pert = (hb @ projection_u[a]) @ projection_v[a]
        out[m] = hb + alpha * pert
    mean = out.mean(-1, keepdims=True, dtype=np.float64)
    var = out.var(-1, keepdims=True, dtype=np.float64)
    out = (out - mean) / np.sqrt(var + LN_EPS)
    return (out * ln_gamma + ln_beta).astype(h.dtype)


def _build_program(nslot, hidden, rank, variant):
    """Build the per-core Bass program. Same program runs SPMD on all 8 cores."""
    from contextlib import ExitStack

    import concourse.bacc as bacc
    import concourse.mybir as mybir
    import concourse.tile as tile

    assert hidden == 1024 and rank == 32
    assert nslot % G == 0
    ngroup = nslot // G
    nchunk = hidden // 128
    T = G * CAP          # tokens per group tile (126)
    KR = G * rank        # stacked rank (96)

    mmdt = mybir.dt.float32r if variant.endswith("r") else mybir.dt.float32

    nc = bacc.Bacc("TRN2", target_bir_lowering=False, debug=False)

    u_d = nc.dram_tensor("u_sw", (ngroup, 128, nchunk * KR), mmdt,
                         kind="ExternalInput")
    v_d = nc.dram_tensor("v_sw", (ngroup, KR, hidden), mmdt,
                         kind="ExternalInput")
    hT_d = nc.dram_tensor("hT_sw", (ngroup, 128, nchunk * T), mmdt,
                          kind="ExternalInput")
    id_d = nc.dram_tensor("ident", (128, 128), mybir.dt.float32,
                          kind="ExternalInput")
    out_d = nc.dram_tensor("out_pk", (ngroup, T, hidden), mybir.dt.float32,
                           kind="ExternalOutput")

    with tile.TileContext(nc) as tc, ExitStack() as ctx:
        upool = ctx.enter_context(tc.tile_pool(name="u", bufs=6))
        vpool = ctx.enter_context(tc.tile_pool(name="v", bufs=6))
        htpool = ctx.enter_context(tc.tile_pool(name="hT", bufs=6))
        bpool = ctx.enter_context(tc.tile_pool(name="blk", bufs=3))
        spool = ctx.enter_context(tc.tile_pool(name="stats", bufs=8))
        opool = ctx.enter_context(tc.tile_pool(name="o", bufs=6))
        cpool = ctx.enter_context(tc.tile_pool(name="const", bufs=1))
        p1pool = ctx.enter_context(tc.tile_pool(name="psum1", bufs=4, space="PSUM"))
        p2pool = ctx.enter_context(tc.tile_pool(name="psum2", bufs=2, space="PSUM"))

        eps_t = cpool.tile([128, 1], mybir.dt.float32)
        nc.vector.memset(eps_t[:], LN_EPS)
        id_t = cpool.tile([128, 128], mybir.dt.float32)
        nc.gpsimd.dma_start(id_t[:], id_d[:])

        for g in range(ngroup):
            u_t = upool.tile([128, nchunk * KR], mmdt)
            hc = nchunk // 2
            nc.scalar.dma_start(u_t[:, 0:hc * KR], u_d[g][:, 0:hc * KR])
            nc.scalar.dma_start(u_t[:, hc * KR:], u_d[g][:, hc * KR:])
            hT_t = htpool.tile([128, nchunk * T], mmdt)
            nc.sync.dma_start(hT_t[:, 0:hc * T], hT_d[g][:, 0:hc * T])
            nc.sync.dma_start(hT_t[:, hc * T:], hT_d[g][:, hc * T:])

            psum1 = p1pool.tile([KR, T], mybir.dt.float32)
            for c in range(nchunk):
                nc.tensor.matmul(
                    psum1[:],
                    u_t[:, c * KR:(c + 1) * KR],
                    hT_t[:, c * T:(c + 1) * T],
                    start=(c == 0), stop=(c == nchunk - 1),
                )

            # block-diag [KR, T] lhsT: diagonal [rank, CAP] blocks from psum1
            blk = bpool.tile([KR, T], mmdt)
            for s in range(G):
                for s2 in range(G):
                    if s == s2:
                        nc.vector.tensor_copy(
                            blk[s * rank:(s + 1) * rank,
                                s2 * CAP:(s2 + 1) * CAP],
                            psum1[s * rank:(s + 1) * rank,
                                  s2 * CAP:(s2 + 1) * CAP])
                    else:
                        nc.gpsimd.memset(
                            blk[s * rank:(s + 1) * rank,
                                s2 * CAP:(s2 + 1) * CAP], 0.0)

            v_t = vpool.tile([KR, hidden], mmdt)
            nc.sync.dma_start(v_t[:], v_d[g])

            psum2 = p2pool.tile([128, hidden], mybir.dt.float32)
            for q in range(hidden // 512):
                nc.tensor.matmul(
                    psum2[0:T, q * 512:(q + 1) * 512],
                    blk[:],
                    v_t[:, q * 512:(q + 1) * 512],
                    start=True, stop=True,
                )
            # accumulate the residual h into psum2 by transposing hT chunks
            # through the PE (x = h + pert materializes in PSUM, no h reload)
            for c in range(nchunk):
                nc.tensor.matmul(
                    psum2[0:T, c * 128:(c + 1) * 128],
                    hT_t[:, c * T:(c + 1) * T],
                    id_t[:],
                    is_transpose=True, start=False, stop=True,
                    skip_group_check=True,
                )

            stats = spool.tile([128, 6 * (hidden // 512)], mybir.dt.float32)
            for q in range(hidden // 512):
                nc.vector.bn_stats(stats[0:T, q * 6:(q + 1) * 6],
                                   psum2[0:T, q * 512:(q + 1) * 512])
            aggr = spool.tile([128, 2], mybir.dt.float32)
            nc.vector.bn_aggr(aggr[0:T, :],
                              stats[0:T, :].rearrange("p (c s) -> p c s", s=3))
            std = spool.tile([128, 1], mybir.dt.float32)
            nc.scalar.activation(std[0:T, :], aggr[0:T, 1:2],
                                 mybir.ActivationFunctionType.Sqrt,
                                 bias=eps_t[0:T, 0:1])
            rstd = spool.tile([128, 1], mybir.dt.float32)
            nc.vector.reciprocal(rstd[0:T, :], std[0:T, :])
            nmr = spool.tile([128, 1], mybir.dt.float32)
            nc.vector.scalar_tensor_tensor(nmr[0:T, :], aggr[0:T, 0:1], -1.0,
                                           rstd[0:T, :],
                                           mybir.AluOpType.mult,
                                           mybir.AluOpType.mult)
            o_t = opool.tile([128, hidden], mybir.dt.float32)
            nc.scalar.activation(o_t[0:T, :], psum2[0:T, :],
                                 mybir.ActivationFunctionType.Identity,
                                 bias=nmr[0:T, 0:1], scale=rstd[0:T, 0:1])
            if g % 2 == 0:
                nc.gpsimd.dma_start(out_d[g], o_t[0:T, :])
            else:
                nc.sync.dma_start(out_d[g], o_t[0:T, :])

    nc.finalize()
    return nc


def _get_program(nslot, hidden, rank, variant):
    key = (nslot, hidden, rank, variant)
    if key not in _PROGRAM_CACHE:
        _PROGRAM_CACHE[key] = _build_program(nslot, hidden, rank, variant)
    return _PROGRAM_CACHE[key]


def kernel(h, log_alpha, ln_gamma, ln_beta, projection_u, projection_v,
           agent_ids):
    h = np.asarray(h, dtype=np.float32)
    projection_u = np.asarray(projection_u, dtype=np.float32)
    projection_v = np.asarray(projection_v, dtype=np.float32)
    ln_gamma = np.asarray(ln_gamma, dtype=np.float32)
    ln_beta = np.asarray(ln_beta, dtype=np.float32)
    ids_raw = np.asarray(agent_ids)
    log_alpha = np.float32(np.asarray(log_alpha))

    B, H = h.shape
    A, H2, R = projection_u.shape
    ids = (ids_raw.astype(np.int64) % A).astype(np.int32)

    if H != 1024 or H2 != H or R != 32 or projection_v.shape != (A, R, H):
        return _reference_numpy(h, log_alpha, ln_gamma, ln_beta, projection_u,
                                projection_v, agent_ids)

    alpha = np.float32(min(np.exp(log_alpha), np.float32(ALPHA_MAX)))
    use_gamma = not np.all(ln_gamma == 1.0)
    use_beta = not np.all(ln_beta == 0.0)

    # ---- host-side MoE dispatch: sort tokens by agent, build capacity slots
    order = np.argsort(ids, kind="stable").astype(np.int64)
    counts = np.bincount(ids, minlength=A)
    starts = np.zeros(A + 1, np.int64)
    np.cumsum(counts, out=starts[1:])

    slot_agent = []   # agent id per slot
    slot_rows = []    # (start, n) into `order` per slot
    for a in range(A):
        n = int(counts[a])
        s = int(starts[a])
        while n > 0:
            take = min(n, CAP)
            slot_agent.append(a)
            slot_rows.append((s, take))
            s += take
            n -= take
    total_slots = len(slot_agent)
    nslot = -(-total_slots // N_CORES)
    nslot = max(nslot, G)
    if nslot % G:
        nslot += G - nslot % G
    if nslot > 96:  # way off the expected distribution; play it safe
        return _reference_numpy(h, log_alpha, ln_gamma, ln_beta, projection_u,
                                projection_v, agent_ids)
    while len(slot_agent) < nslot * N_CORES:
        slot_agent.append(0)
        slot_rows.append((0, 0))
    slot_agent = np.asarray(slot_agent, np.int64)

    ngroup = nslot // G
    nchunk = H // 128
    T = G * CAP
    KR = G * R

    # row_idx: global token index feeding each padded row (clamped for pads)
    nrows = nslot * CAP
    row_idx = np.zeros((N_CORES, nrows), np.int64)
    row_valid = np.zeros((N_CORES, nrows), bool)
    for j, (s, n) in enumerate(slot_rows):
        core, sl = divmod(j, nslot)
        r0 = sl * CAP
        if n:
            row_idx[core, r0:r0 + n] = order[s:s + n]
            row_valid[core, r0:r0 + n] = True

    h_pk = h[row_idx].reshape(N_CORES, ngroup, T, H)
    # hT per group: [p(128), c(8), t(T)]
    hT_sw = np.ascontiguousarray(
        h_pk.reshape(N_CORES, ngroup, T, nchunk, 128)
        .transpose(0, 1, 4, 3, 2)).reshape(N_CORES, ngroup, 128, nchunk * T)
    ident = np.eye(128, dtype=np.float32)

    sa = slot_agent.reshape(N_CORES, nslot)
    # u: [g, p(128), c(8), s(G), r(32)]
    u_sw = np.ascontiguousarray(
        projection_u[sa]                                  # [8, ns, H, R]
        .reshape(N_CORES, ngroup, G, nchunk, 128, R)
        .transpose(0, 1, 4, 3, 2, 5)                      # [8, g, 128, c, G, R]
    ).reshape(N_CORES, ngroup, 128, nchunk * KR)
    v_sw = np.ascontiguousarray(alpha * projection_v[sa]).reshape(
        N_CORES, ngroup, KR, H)

    in_maps = []
    for core in range(N_CORES):
        m = {
            "u_sw": u_sw[core],
            "v_sw": v_sw[core],
            "hT_sw": hT_sw[core],
            "ident": ident,
        }
        in_maps.append(m)

    nc = _get_program(nslot, H, R, VARIANT)

    from concourse.bass_utils import run_bass_kernel_spmd
    res = run_bass_kernel_spmd(nc, in_maps, list(range(N_CORES)))

    out = np.empty_like(h)
    for core in range(N_CORES):
        o = np.asarray(res.results[core]["out_pk"]).reshape(nrows, H)
        out[row_idx[core][row_valid[core]]] = o[row_valid[core]]
    # gamma/beta are applied host-side (the device computes plain LayerNorm);
    # for the common gamma=1/beta=0 case this is a no-op.
    if use_gamma:
        out *= ln_gamma
    if use_beta:
        out += ln_beta
    return out
